# revision 1
# baseline (speedup 1.0000x reference)
"""Linformer attention TRN2 Bass kernel.

Problem: nn_LinformerAttention (B=4, L=4096, D=1024, NH=16, DH=64, k=128).

Sharding: 8 cores = batch(4) x head-group(2). Core c handles batch c%4 and
heads (c//4)*8 .. +8, producing out[b, :, hg*512:(hg+1)*512]. Slices are
disjoint -> no collectives; host reassembles.

Device algorithm per core (all fp32):
  phase 1, streamed over 8 l-chunks of 512:
    - K = x @ Wk.T + bk, V likewise   (PSUM accum over 8 d-subtiles of 128)
    - Q.T = Wq @ x.T + bq (scaled by 1/sqrt(dh) folded into Wq/bq on host),
      spilled to internal DRAM
    - KVp[h] += E_h.T-chunk.T @ [K_h | V_h]  (Linformer projection, both
      [k=128, dh=64], accumulated into SBUF via DVE adds)
  phase 2:
    - KpT[h] = PE-transpose(Kp[h]); Vp_aug[h] = [Vp[h] | ones]
    - dotT[k, l] = KpT.T @ Q.T-chunk   (one matmul per (h, l-chunk))
    - expT = exp(dotT)                 (ACT, no max-subtraction: logits are
                                        small by construction, exp is safe)
    - Xo_aug = expT-tile.T @ Vp_aug -> [l-tile, 65]; col 64 = softmax denom
    - out[:, h*64:+64] = Xo_aug[:, :64] * 1/Xo_aug[:, 64]

Host prep (numpy, outside HW-timed region): x[b].T, W slices pre-transposed
(+1/8 scale on Wq), E head-slices pre-transposed, bias tiles.
"""

import sys

sys.path.insert(0, "/opt/trn_rl_repo")

import math
from contextlib import ExitStack

import numpy as np

import json

import concourse.bass as bass
import concourse.bass2jax as bass2jax
import concourse.mybir as mybir
import concourse.tile as tile
from concourse.bass_utils import compile_bir_kernel as _orig_compile_bir_kernel
from concourse.bass_utils import run_bass_kernel_spmd
from concourse.masks import make_identity


def _split_multiwaits(bir_json_bytes):
    """This container's walrus encodes at most ONE sync wait per engine
    instruction ("Too many sync wait commands" otherwise), while Tile emits
    multi-wait instructions. Hoist extra waits onto single-wait
    EventSemaphore carrier instructions placed just before, on the same
    engine queue — semantically identical stalling."""
    bj = json.loads(bir_json_bytes)
    for fn in bj["functions"]:
        for blk in fn["blocks"]:
            out = []
            for inst in blk["instructions"]:
                si = inst.get("sync_info")
                waits = (si or {}).get("on_wait") or []
                if si and len(waits) > 1:
                    for wi, w in enumerate(waits[:-1]):
                        out.append(
                            {
                                "debug": inst.get("debug", 0),
                                "engine": inst.get("engine"),
                                "ins": [],
                                "outs": [],
                                "name": inst["name"] + "-w%d" % wi,
                                "opcode": "EventSemaphore",
                                "sync_info": {"on_update": [], "on_wait": [w]},
                            }
                        )
                    si["on_wait"] = [waits[-1]]
                out.append(inst)
            blk["instructions"] = out
    return json.dumps(bj).encode()


def _patched_compile_bir_kernel(bir_json, tmpdir, neff_name="file.neff"):
    return _orig_compile_bir_kernel(_split_multiwaits(bir_json), tmpdir, neff_name)


bass2jax.compile_bir_kernel = _patched_compile_bir_kernel

B, L, D = 4, 4096, 1024
NH, DH, KK = 16, 64, 128
NCORES = 8
HGS = 2  # head groups
H = NH // HGS  # 8 local heads per core
J = H * DH  # 512 output columns per core
P = 128
LCH = 512  # l-chunk
NLC = L // LCH  # 8
DC = D // P  # 8 contraction subtiles
JT = J // P  # 4
LT4 = LCH // P  # 4 l-tiles per chunk
F32 = mybir.dt.float32
F32R = mybir.dt.float32r  # full-rate PE matmul, TF32-like product precision

TRACE = False  # test.py sets True to collect a profile
LAST_RESULTS = None  # BassKernelResults of the last kernel() call

_PROGRAM = None


def _build_program():
    nc = bass.Bass()
    xT = nc.declare_dram_parameter("xT", [D, L], F32R, isOutput=False)
    wqT = nc.declare_dram_parameter("wqT", [D, J], F32R, isOutput=False)
    wkT = nc.declare_dram_parameter("wkT", [D, J], F32R, isOutput=False)
    wvT = nc.declare_dram_parameter("wvT", [D, J], F32R, isOutput=False)
    bqT = nc.declare_dram_parameter("bqT", [P, JT], F32, isOutput=False)
    bkB = nc.declare_dram_parameter("bkB", [P, J], F32, isOutput=False)
    bvB = nc.declare_dram_parameter("bvB", [P, J], F32, isOutput=False)
    eT = nc.declare_dram_parameter("eT", [NLC, P, H, LT4, KK], F32, isOutput=False)
    out = nc.declare_dram_parameter("out", [L, J], F32, isOutput=True)
    qtd = nc.dram_tensor("qtd", [J, L], F32R)

    add = mybir.AluOpType.add
    mult = mybir.AluOpType.mult

    with tile.TileContext(nc) as tc:
        with ExitStack() as ctx:
            const = ctx.enter_context(tc.tile_pool(name="const", bufs=1))
            xpool = ctx.enter_context(tc.tile_pool(name="x", bufs=2))
            kvpool = ctx.enter_context(tc.tile_pool(name="kv", bufs=4))
            qtpool = ctx.enter_context(tc.tile_pool(name="qt", bufs=2))
            epool = ctx.enter_context(tc.tile_pool(name="e", bufs=1))
            qthpool = ctx.enter_context(tc.tile_pool(name="qth", bufs=2))
            exppool = ctx.enter_context(tc.tile_pool(name="ex", bufs=3))
            outpool = ctx.enter_context(tc.tile_pool(name="ot", bufs=2))
            recpool = ctx.enter_context(tc.tile_pool(name="rc", bufs=8))
            psA = ctx.enter_context(tc.tile_pool(name="psA", bufs=4, space="PSUM"))
            psB = ctx.enter_context(tc.tile_pool(name="psB", bufs=4, space="PSUM"))

            # ---- constants resident in SBUF
            wq_sb = const.tile([P, DC, J], F32R, tag="wq")
            wk_sb = const.tile([P, DC, J], F32R, tag="wk")
            wv_sb = const.tile([P, DC, J], F32R, tag="wv")
            nc.sync.dma_start(wq_sb[:], wqT[:, :].rearrange("(po pi) j -> pi po j", pi=P))
            nc.sync.dma_start(wk_sb[:], wkT[:, :].rearrange("(po pi) j -> pi po j", pi=P))
            nc.sync.dma_start(wv_sb[:], wvT[:, :].rearrange("(po pi) j -> pi po j", pi=P))
            bqT_sb = const.tile([P, JT], F32, tag="bqT")
            bkB_sb = const.tile([P, J], F32, tag="bkB")
            bvB_sb = const.tile([P, J], F32, tag="bvB")
            nc.sync.dma_start(bqT_sb[:], bqT[:, :])
            nc.sync.dma_start(bkB_sb[:], bkB[:, :])
            nc.sync.dma_start(bvB_sb[:], bvB[:, :])
            ident = const.tile([P, P], F32, tag="ident")
            make_identity(nc, ident[:])

            # Warm-up: make PE observe each weight DMA individually, so no
            # later matmul ever needs two DMA-queue waits at once (the PE
            # Matmult encoding only fits one sync wait -> neuronxcc
            # "Too many sync wait commands" otherwise).
            for wi, w_sb in enumerate((wq_sb, wk_sb, wv_sb)):
                ps_w = psB.tile([1, 1], F32, tag="small", name=f"warm{wi}")
                nc.tensor.matmul(
                    ps_w[:], w_sb[:, 0, 0:1].bitcast(F32),
                    w_sb[:, 0, 0:1].bitcast(F32),
                    start=True, stop=True,
                )
            kvp_acc = [const.tile([P, 2, DH], F32, tag=f"kvp{h}", name=f"kvp{h}") for h in range(H)]
            kpT = [const.tile([DH, KK], F32R, tag=f"kpT{h}", name=f"kpT{h}") for h in range(H)]
            vpa = [const.tile([P, DH + 1], F32, tag=f"vpa{h}", name=f"vpa{h}") for h in range(H)]

            xTr = xT[:, :].rearrange("(po pi) l -> pi po l", pi=P)
            qtdr = qtd[:, :].rearrange("(po pi) l -> pi po l", pi=P)
            outr = out[:, :].rearrange("(lo li) j -> li lo j", li=P)

            # ---- phase 1: projections + Linformer K/V reduction
            for lc in range(NLC):
                x_sb = xpool.tile([P, DC, LCH], F32R, tag="x")
                nc.sync.dma_start(x_sb[:], xTr[:, :, lc * LCH : (lc + 1) * LCH])
                kv_tiles = []
                for lt in range(LT4):
                    psK = psA.tile([P, LCH], F32, tag="big")
                    psV = psA.tile([P, LCH], F32, tag="big")
                    for dc in range(DC):
                        xst = x_sb[:, dc, lt * P : (lt + 1) * P]
                        nc.tensor.matmul(
                            psK[:], xst,
                            wk_sb[:, dc, :],
                            start=(dc == 0), stop=(dc == DC - 1),
                        )
                        nc.tensor.matmul(
                            psV[:], xst,
                            wv_sb[:, dc, :],
                            start=(dc == 0), stop=(dc == DC - 1),
                        )
                    kv_sb = kvpool.tile([P, 2, LCH], F32, tag="kv")
                    nc.any.tensor_tensor(kv_sb[:, 0, :], psK[:], bkB_sb[:], add)
                    nc.any.tensor_tensor(kv_sb[:, 1, :], psV[:], bvB_sb[:], add)
                    kv_tiles.append(kv_sb)
                qt_sb = qtpool.tile([P, JT, LCH], F32R, tag="qt")
                for jt in range(JT):
                    psQ = psA.tile([P, LCH], F32, tag="big")
                    for dc in range(DC):
                        nc.tensor.matmul(
                            psQ[:], wq_sb[:, dc, jt * P : (jt + 1) * P],
                            x_sb[:, dc, :],
                            start=(dc == 0), stop=(dc == DC - 1),
                        )
                    nc.any.tensor_scalar(
                        qt_sb[:, jt, :], psQ[:], bqT_sb[:, jt : jt + 1], None, add
                    )
                nc.sync.dma_start(
                    qtdr[:, :, lc * LCH : (lc + 1) * LCH], qt_sb[:]
                )
                e_sb = epool.tile([P, H, LT4, KK], F32, tag="e")
                nc.sync.dma_start(e_sb[:], eT[lc])
                for h in range(H):
                    psKV = psB.tile([P, 2, DH], F32, tag="small")
                    for lt in range(LT4):
                        nc.tensor.matmul(
                            psKV[:], e_sb[:, h, lt, :],
                            kv_tiles[lt][:, :, h * DH : (h + 1) * DH],
                            start=(lt == 0), stop=(lt == LT4 - 1),
                        )
                    if lc == 0:
                        nc.any.tensor_copy(kvp_acc[h][:], psKV[:])
                    else:
                        nc.any.tensor_tensor(
                            kvp_acc[h][:], kvp_acc[h][:], psKV[:], add
                        )

            # ---- phase 2: attention
            for h in range(H):
                psT = psB.tile([DH, KK], F32, tag="small")
                nc.tensor.transpose(psT[:], kvp_acc[h][:, 0, :], ident[:])
                nc.any.tensor_copy(kpT[h][:], psT[:])
                nc.any.tensor_copy(vpa[h][:, 0:DH], kvp_acc[h][:, 1, :])
                nc.any.memset(vpa[h][:, DH : DH + 1], 1.0)

            for lc in range(NLC):
                qtc = qthpool.tile([DH, 2 * JT, LCH], F32R, tag="qth")
                nc.sync.dma_start(
                    qtc[:],
                    qtd[:, lc * LCH : (lc + 1) * LCH].rearrange(
                        "(h dh) l -> dh h l", dh=DH
                    ),
                )
                ot = outpool.tile([P, LT4, J], F32, tag="ot")
                for h in range(H):
                    qth = qtc[:, h, :]
                    psD = psA.tile([P, LCH], F32, tag="big")
                    nc.tensor.matmul(
                        psD[:], kpT[h][:], qth,
                        start=True, stop=True,
                    )
                    ex = exppool.tile([P, LCH], F32, tag="ex")
                    nc.scalar.activation(
                        ex[:], psD[:], mybir.ActivationFunctionType.Exp
                    )
                    for lt in range(LT4):
                        psX = psB.tile([P, DH + 1], F32, tag="small")
                        nc.tensor.matmul(
                            psX[:], ex[:, lt * P : (lt + 1) * P], vpa[h][:],
                            start=True, stop=True,
                        )
                        rc = recpool.tile([P, 1], F32, tag="rc")
                        nc.vector.reciprocal(rc[:], psX[:, DH : DH + 1])
                        nc.any.tensor_tensor(
                            ot[:, lt, h * DH : (h + 1) * DH],
                            psX[:, 0:DH],
                            rc[:].to_broadcast([P, DH]),
                            mult,
                        )
                nc.sync.dma_start(
                    outr[:, lc * LT4 : (lc + 1) * LT4, :], ot[:]
                )

    return nc


def _get_program():
    global _PROGRAM
    if _PROGRAM is None:
        _PROGRAM = _build_program()
    return _PROGRAM


def kernel(x, Wq, bq, Wk, bk, Wv, bv, E):
    global LAST_RESULTS
    x = np.ascontiguousarray(np.asarray(x, dtype=np.float32))
    Wq = np.asarray(Wq, dtype=np.float32)
    bq = np.asarray(bq, dtype=np.float32)
    Wk = np.asarray(Wk, dtype=np.float32)
    bk = np.asarray(bk, dtype=np.float32)
    Wv = np.asarray(Wv, dtype=np.float32)
    bv = np.asarray(bv, dtype=np.float32)
    E = np.asarray(E, dtype=np.float32)

    scale = 1.0 / math.sqrt(DH)
    xTs = [np.ascontiguousarray(x[b].T) for b in range(B)]
    in_maps = []
    for core in range(NCORES):
        b = core % B
        hg = core // B
        js = slice(hg * J, (hg + 1) * J)
        hs = slice(hg * H, (hg + 1) * H)
        wqTs = np.ascontiguousarray((Wq[js, :] * scale).T)
        wkTs = np.ascontiguousarray(Wk[js, :].T)
        wvTs = np.ascontiguousarray(Wv[js, :].T)
        bqTs = np.ascontiguousarray((bq[js] * scale).reshape(JT, P).T)
        bkBs = np.ascontiguousarray(np.broadcast_to(bk[js], (P, J)))
        bvBs = np.ascontiguousarray(np.broadcast_to(bv[js], (P, J)))
        E_s = E[hs]  # [H, KK, L]
        eTs = np.ascontiguousarray(
            E_s.reshape(H, KK, NLC, LT4, P).transpose(2, 4, 0, 3, 1)
        )  # [NLC, P, H, LT4, KK]
        in_maps.append(
            {
                "xT": xTs[b],
                "wqT": wqTs,
                "wkT": wkTs,
                "wvT": wvTs,
                "bqT": bqTs,
                "bkB": bkBs,
                "bvB": bvBs,
                "eT": eTs,
            }
        )

    nc = _get_program()
    res = run_bass_kernel_spmd(nc, in_maps, list(range(NCORES)), trace=TRACE)
    LAST_RESULTS = res

    outp = np.empty((B, L, D), dtype=np.float32)
    for core in range(NCORES):
        b = core % B
        hg = core // B
        outp[b, :, hg * J : (hg + 1) * J] = res.results[core]["out"]
    return outp



# revision 5
# speedup vs baseline: 1.5542x; 1.5542x over previous
"""Linformer attention TRN2 Bass kernel (v2).

Problem: nn_LinformerAttention (B=4, L=4096, D=1024, NH=16, DH=64, k=128).

Sharding: 8 cores = batch(4) x head-group(2). Core c handles batch c%4 and
heads (c//4)*8 .. +8, producing out[b, :, hg*512:(hg+1)*512]. Slices are
disjoint -> no collectives; host reassembles.

Device algorithm per core:
  phase 1, streamed over 8 l-chunks of 512:
    - K = x @ Wk.T + bk, V likewise (fp32r matmuls, PSUM accum over 8
      d-subtiles of 128; bias added on DVE, K/V tiles written as bf16)
    - Q.T = Wq @ x.T + bq (scaled by 1/sqrt(dh) folded into Wq/bq on host),
      written bf16 and kept RESIDENT in SBUF for phase 2 (no DRAM spill)
    - KVp[h] += E_h-chunk @ [K_h | V_h]  (bf16 matmuls, full-rate;
      accumulated into SBUF fp32 via DVE adds)
  phase 2:
    - KpT[h] = PE-transpose(Kp[h]) -> bf16; Vp_aug[h] = [Vp[h] | ones] bf16
    - dotT[k, l] = KpT.T @ Q.T-chunk  (bf16; Q head slice read at partition
      offset (h%2)*64 straight out of the resident qt tile)
    - expT = exp(dotT) -> bf16       (ACT; logits are small by construction,
                                      exp without max-subtraction is safe)
    - psX[lt] = expT-tile.T @ Vp_aug -> [4, l-tile, 65] in ONE PSUM tile;
      col 64 = softmax denominator
    - one strided reciprocal + one fused broadcast-multiply normalizes all
      4 l-tiles of a head at once
  DMA: x-chunk0 + Wk emitted before the other weights so the first matmul
  starts ~12us in instead of waiting for the whole constant batch.

Host prep (numpy, outside HW-timed region): x[b].T, W slices pre-transposed
(+1/8 scale on Wq), E head-slices pre-transposed and cast to bf16, bias
tiles.
"""

import sys

sys.path.insert(0, "/opt/trn_rl_repo")

import math
from contextlib import ExitStack

import numpy as np

import json

import concourse.bass as bass
import concourse.bass2jax as bass2jax
import concourse.mybir as mybir
import concourse.tile as tile
from concourse.bass_utils import compile_bir_kernel as _orig_compile_bir_kernel
from concourse.bass_utils import run_bass_kernel_spmd
from concourse.masks import make_identity


def _split_multiwaits(bir_json_bytes):
    """This container's walrus encodes at most ONE sync wait per engine
    instruction ("Too many sync wait commands" otherwise), while Tile emits
    multi-wait instructions. Hoist extra waits onto single-wait
    EventSemaphore carrier instructions placed just before, on the same
    engine queue — semantically identical stalling."""
    bj = json.loads(bir_json_bytes)
    for fn in bj["functions"]:
        for blk in fn["blocks"]:
            out = []
            for inst in blk["instructions"]:
                si = inst.get("sync_info")
                waits = (si or {}).get("on_wait") or []
                if si and len(waits) > 1:
                    for wi, w in enumerate(waits[:-1]):
                        out.append(
                            {
                                "debug": inst.get("debug", 0),
                                "engine": inst.get("engine"),
                                "ins": [],
                                "outs": [],
                                "name": inst["name"] + "-w%d" % wi,
                                "opcode": "EventSemaphore",
                                "sync_info": {"on_update": [], "on_wait": [w]},
                            }
                        )
                    si["on_wait"] = [waits[-1]]
                out.append(inst)
            blk["instructions"] = out
    return json.dumps(bj).encode()


def _patched_compile_bir_kernel(bir_json, tmpdir, neff_name="file.neff"):
    return _orig_compile_bir_kernel(_split_multiwaits(bir_json), tmpdir, neff_name)


bass2jax.compile_bir_kernel = _patched_compile_bir_kernel

B, L, D = 4, 4096, 1024
NH, DH, KK = 16, 64, 128
NCORES = 8
HGS = 2  # head groups
H = NH // HGS  # 8 local heads per core
J = H * DH  # 512 output columns per core
P = 128
LCH = 512  # l-chunk
NLC = L // LCH  # 8
DC = D // P  # 8 contraction subtiles
JT = J // P  # 4
LT4 = LCH // P  # 4 l-tiles per chunk
F32 = mybir.dt.float32
F32R = mybir.dt.float32r  # full-rate PE matmul, TF32-like product precision
BF16 = mybir.dt.bfloat16

TRACE = False  # test.py sets True to collect a profile
LAST_RESULTS = None  # BassKernelResults of the last kernel() call

_PROGRAM = None


def _build_program():
    nc = bass.Bass()
    xT = nc.declare_dram_parameter("xT", [D, L], F32R, isOutput=False)
    wqT = nc.declare_dram_parameter("wqT", [D, J], F32R, isOutput=False)
    wkT = nc.declare_dram_parameter("wkT", [D, J], F32R, isOutput=False)
    wvT = nc.declare_dram_parameter("wvT", [D, J], F32R, isOutput=False)
    bqT = nc.declare_dram_parameter("bqT", [P, JT], F32, isOutput=False)
    bkB = nc.declare_dram_parameter("bkB", [P, J], F32, isOutput=False)
    bvB = nc.declare_dram_parameter("bvB", [P, J], F32, isOutput=False)
    eT = nc.declare_dram_parameter("eT", [NLC, P, H, LT4, KK], BF16, isOutput=False)
    out = nc.declare_dram_parameter("out", [L, J], F32, isOutput=True)

    add = mybir.AluOpType.add
    mult = mybir.AluOpType.mult

    with tile.TileContext(nc) as tc:
        with ExitStack() as ctx:
            const = ctx.enter_context(tc.tile_pool(name="const", bufs=1))
            xpool = ctx.enter_context(tc.tile_pool(name="x", bufs=2))
            kvpool = ctx.enter_context(tc.tile_pool(name="kv", bufs=4))
            epool = ctx.enter_context(tc.tile_pool(name="e", bufs=2))
            exppool = ctx.enter_context(tc.tile_pool(name="ex", bufs=3))
            outpool = ctx.enter_context(tc.tile_pool(name="ot", bufs=2))
            recpool = ctx.enter_context(tc.tile_pool(name="rc", bufs=4))
            psA = ctx.enter_context(tc.tile_pool(name="psA", bufs=4, space="PSUM"))
            psB = ctx.enter_context(tc.tile_pool(name="psB", bufs=4, space="PSUM"))

            xTr = xT[:, :].rearrange("(po pi) l -> pi po l", pi=P)
            outr = out[:, :].rearrange("(lo li) j -> li lo j", li=P)

            # ---- first x chunk + Wk first: the DMA engines stripe the whole
            # emitted batch round-robin, so what the first matmul needs must
            # head the queue.
            x_cur = xpool.tile([P, DC, LCH], F32R, tag="x")
            nc.sync.dma_start(x_cur[:], xTr[:, :, 0:LCH])
            wk_sb = const.tile([P, DC, J], F32R, tag="wk")
            nc.sync.dma_start(wk_sb[:], wkT[:, :].rearrange("(po pi) j -> pi po j", pi=P))
            wv_sb = const.tile([P, DC, J], F32R, tag="wv")
            nc.sync.dma_start(wv_sb[:], wvT[:, :].rearrange("(po pi) j -> pi po j", pi=P))
            wq_sb = const.tile([P, DC, J], F32R, tag="wq")
            nc.sync.dma_start(wq_sb[:], wqT[:, :].rearrange("(po pi) j -> pi po j", pi=P))
            bkB_sb = const.tile([P, J], F32, tag="bkB")
            bvB_sb = const.tile([P, J], F32, tag="bvB")
            bqT_sb = const.tile([P, JT], F32, tag="bqT")
            nc.sync.dma_start(bkB_sb[:], bkB[:, :])
            nc.sync.dma_start(bvB_sb[:], bvB[:, :])
            nc.sync.dma_start(bqT_sb[:], bqT[:, :])
            e_cur = epool.tile([P, H, LT4, KK], BF16, tag="e")
            nc.sync.dma_start(e_cur[:], eT[0])
            ident = const.tile([P, P], F32, tag="ident")
            make_identity(nc, ident[:])

            # Warm-up: make PE observe each weight DMA individually, so no
            # later matmul ever needs two DMA-queue waits at once (the PE
            # Matmult encoding only fits one sync wait -> neuronxcc
            # "Too many sync wait commands" otherwise).
            for wi, w_sb in enumerate((wk_sb, wv_sb, wq_sb)):
                ps_w = psB.tile([1, 1], F32, tag="small", name=f"warm{wi}")
                nc.tensor.matmul(
                    ps_w[:], w_sb[:, 0, 0:1].bitcast(F32),
                    w_sb[:, 0, 0:1].bitcast(F32),
                    start=True, stop=True,
                )

            # Per head PAIR (jt): [k, K|V, head-in-pair, dh] so that one PE
            # transpose of the K plane puts head 2jt's KpT on partitions
            # 0..63 and head 2jt+1's on 64..127 — matching the partition
            # offset of the resident Q slices (matmul requires equal base
            # partitions for stationary and moving operands).
            kvp_acc = [const.tile([P, 2, 2, DH], F32, tag=f"kvp{j}", name=f"kvp{j}") for j in range(JT)]
            kpp = [const.tile([P, KK], BF16, tag=f"kpp{j}", name=f"kpp{j}") for j in range(JT)]
            vpa = [const.tile([P, DH + 1], BF16, tag=f"vpa{h}", name=f"vpa{h}") for h in range(H)]
            # Q.T resident in SBUF, bf16: [dh|dh, lc, jt, l]; head h of chunk
            # lc lives at partition offset (h%2)*64, plane jt=h//2.
            qt_all = const.tile([P, NLC, JT, LCH], BF16, tag="qt")

            # ---- phase 1: projections + Linformer K/V reduction
            for lc in range(NLC):
                x_sb, e_sb = x_cur, e_cur
                if lc + 1 < NLC:
                    x_cur = xpool.tile([P, DC, LCH], F32R, tag="x")
                    nc.sync.dma_start(
                        x_cur[:], xTr[:, :, (lc + 1) * LCH : (lc + 2) * LCH]
                    )
                    e_cur = epool.tile([P, H, LT4, KK], BF16, tag="e")
                    nc.sync.dma_start(e_cur[:], eT[lc + 1])
                kv_tiles = []
                for lt in range(LT4):
                    psK = psA.tile([P, LCH], F32, tag="big")
                    psV = psA.tile([P, LCH], F32, tag="big")
                    for dc in range(DC):
                        xst = x_sb[:, dc, lt * P : (lt + 1) * P]
                        nc.tensor.matmul(
                            psK[:], xst,
                            wk_sb[:, dc, :],
                            start=(dc == 0), stop=(dc == DC - 1),
                        )
                        nc.tensor.matmul(
                            psV[:], xst,
                            wv_sb[:, dc, :],
                            start=(dc == 0), stop=(dc == DC - 1),
                        )
                    kv_sb = kvpool.tile([P, 2, LCH], BF16, tag="kv")
                    nc.any.tensor_tensor(kv_sb[:, 0, :], psK[:], bkB_sb[:], add)
                    nc.any.tensor_tensor(kv_sb[:, 1, :], psV[:], bvB_sb[:], add)
                    kv_tiles.append(kv_sb)
                for jt in range(JT):
                    psQ = psA.tile([P, LCH], F32, tag="big")
                    for dc in range(DC):
                        nc.tensor.matmul(
                            psQ[:], wq_sb[:, dc, jt * P : (jt + 1) * P],
                            x_sb[:, dc, :],
                            start=(dc == 0), stop=(dc == DC - 1),
                        )
                    nc.any.tensor_scalar(
                        qt_all[:, lc, jt, :], psQ[:], bqT_sb[:, jt : jt + 1], None, add
                    )
                for h in range(H):
                    psKV = psB.tile([P, 2, DH], F32, tag="small")
                    for lt in range(LT4):
                        nc.tensor.matmul(
                            psKV[:], e_sb[:, h, lt, :],
                            kv_tiles[lt][:, :, h * DH : (h + 1) * DH],
                            start=(lt == 0), stop=(lt == LT4 - 1),
                        )
                    acc = kvp_acc[h // 2][:, :, h % 2, :]
                    if lc == 0:
                        nc.any.tensor_copy(acc, psKV[:])
                    else:
                        nc.any.tensor_tensor(acc, acc, psKV[:], add)

            # ---- phase 2: attention
            for j in range(JT):
                psT = psB.tile([P, KK], F32, tag="small")
                nc.tensor.transpose(
                    psT[:], kvp_acc[j][:, 0, :, :], ident[:]
                )
                nc.any.tensor_copy(kpp[j][:], psT[:])
            for h in range(H):
                nc.any.tensor_copy(
                    vpa[h][:, 0:DH], kvp_acc[h // 2][:, 1, h % 2, :]
                )
                nc.any.memset(vpa[h][:, DH : DH + 1], 1.0)

            for lc in range(NLC):
                ot = outpool.tile([P, LT4, J], F32, tag="ot")
                for h in range(H):
                    qth = qt_all[(h % 2) * DH : (h % 2 + 1) * DH, lc, h // 2, :]
                    kph = kpp[h // 2][(h % 2) * DH : (h % 2 + 1) * DH, :]
                    psD = psA.tile([P, LCH], F32, tag="big")
                    nc.tensor.matmul(
                        psD[:], kph, qth,
                        start=True, stop=True,
                    )
                    ex = exppool.tile([P, LCH], BF16, tag="ex")
                    nc.scalar.activation(
                        ex[:], psD[:], mybir.ActivationFunctionType.Exp
                    )
                    psX = psB.tile([P, LT4, DH + 1], F32, tag="small")
                    for lt in range(LT4):
                        nc.tensor.matmul(
                            psX[:, lt, :], ex[:, lt * P : (lt + 1) * P], vpa[h][:],
                            start=True, stop=True,
                        )
                    rc = recpool.tile([P, LT4], F32, tag="rc")
                    nc.vector.reciprocal(rc[:], psX[:, :, DH])
                    nc.any.tensor_tensor(
                        ot[:, :, h * DH : (h + 1) * DH],
                        psX[:, :, 0:DH],
                        rc[:].to_broadcast([P, LT4, DH]),
                        mult,
                    )
                nc.sync.dma_start(
                    outr[:, lc * LT4 : (lc + 1) * LT4, :], ot[:]
                )

    return nc


def _get_program():
    global _PROGRAM
    if _PROGRAM is None:
        _PROGRAM = _build_program()
    return _PROGRAM


def kernel(x, Wq, bq, Wk, bk, Wv, bv, E):
    global LAST_RESULTS
    x = np.ascontiguousarray(np.asarray(x, dtype=np.float32))
    Wq = np.asarray(Wq, dtype=np.float32)
    bq = np.asarray(bq, dtype=np.float32)
    Wk = np.asarray(Wk, dtype=np.float32)
    bk = np.asarray(bk, dtype=np.float32)
    Wv = np.asarray(Wv, dtype=np.float32)
    bv = np.asarray(bv, dtype=np.float32)
    E = np.asarray(E, dtype=np.float32)

    bf16 = mybir.dt.np(BF16)
    scale = 1.0 / math.sqrt(DH)
    xTs = [np.ascontiguousarray(x[b].T) for b in range(B)]
    in_maps = []
    for core in range(NCORES):
        b = core % B
        hg = core // B
        js = slice(hg * J, (hg + 1) * J)
        hs = slice(hg * H, (hg + 1) * H)
        wqTs = np.ascontiguousarray((Wq[js, :] * scale).T)
        wkTs = np.ascontiguousarray(Wk[js, :].T)
        wvTs = np.ascontiguousarray(Wv[js, :].T)
        bqTs = np.ascontiguousarray((bq[js] * scale).reshape(JT, P).T)
        bkBs = np.ascontiguousarray(np.broadcast_to(bk[js], (P, J)))
        bvBs = np.ascontiguousarray(np.broadcast_to(bv[js], (P, J)))
        E_s = E[hs]  # [H, KK, L]
        eTs = np.ascontiguousarray(
            E_s.reshape(H, KK, NLC, LT4, P).transpose(2, 4, 0, 3, 1).astype(bf16)
        )  # [NLC, P, H, LT4, KK] bf16
        in_maps.append(
            {
                "xT": xTs[b],
                "wqT": wqTs,
                "wkT": wkTs,
                "wvT": wvTs,
                "bqT": bqTs,
                "bkB": bkBs,
                "bvB": bvBs,
                "eT": eTs,
            }
        )

    nc = _get_program()
    res = run_bass_kernel_spmd(nc, in_maps, list(range(NCORES)), trace=TRACE)
    LAST_RESULTS = res

    outp = np.empty((B, L, D), dtype=np.float32)
    for core in range(NCORES):
        b = core % B
        hg = core // B
        outp[b, :, hg * J : (hg + 1) * J] = res.results[core]["out"]
    return outp


# revision 11
# speedup vs baseline: 1.6762x; 1.0785x over previous
"""Linformer attention TRN2 Bass kernel (v2).

Problem: nn_LinformerAttention (B=4, L=4096, D=1024, NH=16, DH=64, k=128).

Sharding: 8 cores = batch(4) x head-group(2). Core c handles batch c%4 and
heads (c//4)*8 .. +8, producing out[b, :, hg*512:(hg+1)*512]. Slices are
disjoint -> no collectives; host reassembles.

Device algorithm per core:
  phase 1, streamed over 8 l-chunks of 512:
    - K = x @ Wk.T + bk, V likewise (fp32r matmuls, PSUM accum over 8
      d-subtiles of 128; bias added on DVE, K/V tiles written as bf16)
    - Q.T = Wq @ x.T + bq (scaled by 1/sqrt(dh) folded into Wq/bq on host),
      written bf16 and kept RESIDENT in SBUF for phase 2 (no DRAM spill)
    - KVp[h] += E_h-chunk @ [K_h | V_h]  (bf16 matmuls, full-rate;
      accumulated into SBUF fp32 via DVE adds)
  phase 2:
    - KpT[h] = PE-transpose(Kp[h]) -> bf16; Vp_aug[h] = [Vp[h] | ones] bf16
    - dotT[k, l] = KpT.T @ Q.T-chunk  (bf16; Q head slice read at partition
      offset (h%2)*64 straight out of the resident qt tile)
    - expT = exp(dotT) -> bf16       (ACT; logits are small by construction,
                                      exp without max-subtraction is safe)
    - psX[lt] = expT-tile.T @ Vp_aug -> [4, l-tile, 65] in ONE PSUM tile;
      col 64 = softmax denominator
    - one strided reciprocal + one fused broadcast-multiply normalizes all
      4 l-tiles of a head at once
  DMA: x-chunk0 + Wk emitted before the other weights so the first matmul
  starts ~12us in instead of waiting for the whole constant batch.

Host prep (numpy, outside HW-timed region): x[b].T, W slices pre-transposed
(+1/8 scale on Wq), E head-slices pre-transposed and cast to bf16, bias
tiles.
"""

import sys

sys.path.insert(0, "/opt/trn_rl_repo")

import math
from contextlib import ExitStack

import numpy as np

import json

import concourse.bass as bass
import concourse.bass2jax as bass2jax
import concourse.mybir as mybir
import concourse.tile as tile
from concourse.bass_utils import compile_bir_kernel as _orig_compile_bir_kernel
from concourse.bass_utils import run_bass_kernel_spmd
from concourse.masks import make_identity


def _split_multiwaits(bir_json_bytes):
    """This container's walrus encodes at most ONE sync wait per engine
    instruction ("Too many sync wait commands" otherwise), while Tile emits
    multi-wait instructions. Hoist extra waits onto single-wait
    EventSemaphore carrier instructions placed just before, on the same
    engine queue — semantically identical stalling."""
    bj = json.loads(bir_json_bytes)
    for fn in bj["functions"]:
        for blk in fn["blocks"]:
            out = []
            for inst in blk["instructions"]:
                si = inst.get("sync_info")
                waits = (si or {}).get("on_wait") or []
                if si and len(waits) > 1:
                    for wi, w in enumerate(waits[:-1]):
                        out.append(
                            {
                                "debug": inst.get("debug", 0),
                                "engine": inst.get("engine"),
                                "ins": [],
                                "outs": [],
                                "name": inst["name"] + "-w%d" % wi,
                                "opcode": "EventSemaphore",
                                "sync_info": {"on_update": [], "on_wait": [w]},
                            }
                        )
                    si["on_wait"] = [waits[-1]]
                out.append(inst)
            blk["instructions"] = out
    return json.dumps(bj).encode()


def _patched_compile_bir_kernel(bir_json, tmpdir, neff_name="file.neff"):
    return _orig_compile_bir_kernel(_split_multiwaits(bir_json), tmpdir, neff_name)


bass2jax.compile_bir_kernel = _patched_compile_bir_kernel

B, L, D = 4, 4096, 1024
NH, DH, KK = 16, 64, 128
NCORES = 8
HGS = 2  # head groups
H = NH // HGS  # 8 local heads per core
J = H * DH  # 512 output columns per core
P = 128
LCH = 512  # l-chunk
NLC = L // LCH  # 8
DC = D // P  # 8 contraction subtiles
JT = J // P  # 4
LT4 = LCH // P  # 4 l-tiles per chunk
F32 = mybir.dt.float32
F32R = mybir.dt.float32r  # full-rate PE matmul, TF32-like product precision
BF16 = mybir.dt.bfloat16

TRACE = False  # test.py sets True to collect a profile
LAST_RESULTS = None  # BassKernelResults of the last kernel() call

_PROGRAM = None


def _build_program():
    nc = bass.Bass()
    xT = nc.declare_dram_parameter("xT", [D, L], BF16, isOutput=False)
    wqT = nc.declare_dram_parameter("wqT", [D, J], BF16, isOutput=False)
    wkT = nc.declare_dram_parameter("wkT", [D, J], BF16, isOutput=False)
    wvT = nc.declare_dram_parameter("wvT", [D, J], BF16, isOutput=False)
    bqT = nc.declare_dram_parameter("bqT", [P, JT], F32, isOutput=False)
    bkB = nc.declare_dram_parameter("bkB", [P, J], F32, isOutput=False)
    bvB = nc.declare_dram_parameter("bvB", [P, J], F32, isOutput=False)
    eT = nc.declare_dram_parameter("eT", [NLC, P, H, LT4, KK], BF16, isOutput=False)
    out = nc.declare_dram_parameter("out", [L, J], F32, isOutput=True)

    add = mybir.AluOpType.add
    mult = mybir.AluOpType.mult

    with tile.TileContext(nc) as tc:
        with ExitStack() as ctx:
            const = ctx.enter_context(tc.tile_pool(name="const", bufs=1))
            xpool = ctx.enter_context(tc.tile_pool(name="x", bufs=2))
            kvpool = ctx.enter_context(tc.tile_pool(name="kv", bufs=4))
            epool = ctx.enter_context(tc.tile_pool(name="e", bufs=2))
            exppool = ctx.enter_context(tc.tile_pool(name="ex", bufs=3))
            outpool = ctx.enter_context(tc.tile_pool(name="ot", bufs=2))
            recpool = ctx.enter_context(tc.tile_pool(name="rc", bufs=4))
            psA = ctx.enter_context(tc.tile_pool(name="psA", bufs=4, space="PSUM"))
            psB = ctx.enter_context(tc.tile_pool(name="psB", bufs=4, space="PSUM"))

            xTr = xT[:, :].rearrange("(po pi) l -> pi po l", pi=P)
            outr = out[:, :].rearrange("(lo li) j -> li lo j", li=P)

            # ---- first x chunk + Wk + Wv first: the DMA engines stripe the
            # emitted batch in order, so what the first matmuls need must
            # head the queue. Everything is bf16 so the critical prefix
            # (x0+wk+wv ~ 3.2 MB) lands in ~9 us.
            x_cur = xpool.tile([P, DC, LCH], BF16, tag="x")
            nc.sync.dma_start(x_cur[:], xTr[:, :, 0:LCH])
            wk_sb = const.tile([P, DC, J], BF16, tag="wk")
            nc.sync.dma_start(wk_sb[:], wkT[:, :].rearrange("(po pi) j -> pi po j", pi=P))
            wv_sb = const.tile([P, DC, J], BF16, tag="wv")
            nc.sync.dma_start(wv_sb[:], wvT[:, :].rearrange("(po pi) j -> pi po j", pi=P))
            wq_sb = const.tile([P, DC, J], BF16, tag="wq")
            nc.sync.dma_start(wq_sb[:], wqT[:, :].rearrange("(po pi) j -> pi po j", pi=P))
            e_cur = epool.tile([P, H, LT4, KK], BF16, tag="e")
            nc.sync.dma_start(e_cur[:], eT[0])
            bkB_sb = const.tile([P, J], F32, tag="bkB")
            bvB_sb = const.tile([P, J], F32, tag="bvB")
            bqT_sb = const.tile([P, JT], F32, tag="bqT")
            nc.sync.dma_start(bkB_sb[:], bkB[:, :])
            nc.sync.dma_start(bvB_sb[:], bvB[:, :])
            nc.sync.dma_start(bqT_sb[:], bqT[:, :])
            ident = const.tile([P, P], F32, tag="ident")
            make_identity(nc, ident[:])

            # Warm-up: make PE observe each weight DMA individually, so no
            # later matmul ever needs two DMA-queue waits at once (the PE
            # Matmult encoding only fits one sync wait -> neuronxcc
            # "Too many sync wait commands" otherwise). wq's warm-up is
            # deferred until just before the first Q matmul so the K/V
            # stream isn't gated on the wq DMA.
            def warm(w_sb, wi):
                ps_w = psB.tile([1, 1], F32, tag="small", name=f"warm{wi}")
                nc.tensor.matmul(
                    ps_w[:], w_sb[:, 0, 0:1].bitcast(BF16),
                    w_sb[:, 0, 0:1].bitcast(BF16),
                    start=True, stop=True,
                )

            warm(wk_sb, 0)
            warm(wv_sb, 1)

            # Per head PAIR (jt): [k, K|V, head-in-pair, dh] so that one PE
            # transpose of the K plane puts head 2jt's KpT on partitions
            # 0..63 and head 2jt+1's on 64..127 — matching the partition
            # offset of the resident Q slices (matmul requires equal base
            # partitions for stationary and moving operands).
            kvp_acc = [const.tile([P, 2, 2, DH], F32, tag=f"kvp{j}", name=f"kvp{j}") for j in range(JT)]
            kpp = [const.tile([P, KK], BF16, tag=f"kpp{j}", name=f"kpp{j}") for j in range(JT)]
            vpa = [const.tile([P, DH + 1], BF16, tag=f"vpa{h}", name=f"vpa{h}") for h in range(H)]
            # Q.T resident in SBUF, bf16: [dh|dh, lc, jt, l]; head h of chunk
            # lc lives at partition offset (h%2)*64, plane jt=h//2.
            qt_all = const.tile([P, NLC, JT, LCH], BF16, tag="qt")

            # ---- phase 1: projections + Linformer K/V reduction
            for lc in range(NLC):
                x_sb, e_sb = x_cur, e_cur
                if lc + 1 < NLC:
                    x_cur = xpool.tile([P, DC, LCH], BF16, tag="x")
                    nc.sync.dma_start(
                        x_cur[:], xTr[:, :, (lc + 1) * LCH : (lc + 2) * LCH]
                    )
                    e_cur = epool.tile([P, H, LT4, KK], BF16, tag="e")
                    nc.sync.dma_start(e_cur[:], eT[lc + 1])
                kv_tiles = []
                for lt in range(LT4):
                    psK = psA.tile([P, LCH], F32, tag="big")
                    psV = psA.tile([P, LCH], F32, tag="big")
                    for dc in range(DC):
                        xst = x_sb[:, dc, lt * P : (lt + 1) * P]
                        nc.tensor.matmul(
                            psK[:], xst,
                            wk_sb[:, dc, :],
                            start=(dc == 0), stop=(dc == DC - 1),
                        )
                        nc.tensor.matmul(
                            psV[:], xst,
                            wv_sb[:, dc, :],
                            start=(dc == 0), stop=(dc == DC - 1),
                        )
                    kv_sb = kvpool.tile([P, 2, LCH], BF16, tag="kv")
                    nc.any.tensor_tensor(kv_sb[:, 0, :], psK[:], bkB_sb[:], add)
                    nc.any.tensor_tensor(kv_sb[:, 1, :], psV[:], bvB_sb[:], add)
                    kv_tiles.append(kv_sb)
                if lc == 0:
                    warm(wq_sb, 2)
                for jt in range(JT):
                    psQ = psA.tile([P, LCH], F32, tag="big")
                    for dc in range(DC):
                        nc.tensor.matmul(
                            psQ[:], wq_sb[:, dc, jt * P : (jt + 1) * P],
                            x_sb[:, dc, :],
                            start=(dc == 0), stop=(dc == DC - 1),
                        )
                    nc.any.tensor_scalar(
                        qt_all[:, lc, jt, :], psQ[:], bqT_sb[:, jt : jt + 1], None, add
                    )
                for h in range(H):
                    psKV = psB.tile([P, 2, DH], F32, tag="small")
                    for lt in range(LT4):
                        nc.tensor.matmul(
                            psKV[:], e_sb[:, h, lt, :],
                            kv_tiles[lt][:, :, h * DH : (h + 1) * DH],
                            start=(lt == 0), stop=(lt == LT4 - 1),
                        )
                    acc = kvp_acc[h // 2][:, :, h % 2, :]
                    if lc == 0:
                        nc.any.tensor_copy(acc, psKV[:])
                    else:
                        nc.any.tensor_tensor(acc, acc, psKV[:], add)

            # ---- phase 2: attention
            for j in range(JT):
                psT = psB.tile([P, KK], F32, tag="small")
                nc.tensor.transpose(
                    psT[:], kvp_acc[j][:, 0, :, :], ident[:]
                )
                nc.any.tensor_copy(kpp[j][:], psT[:])
            for h in range(H):
                nc.any.tensor_copy(
                    vpa[h][:, 0:DH], kvp_acc[h // 2][:, 1, h % 2, :]
                )
                nc.any.memset(vpa[h][:, DH : DH + 1], 1.0)

            # psD runs two heads ahead of psX: the PE queue is in-order, so
            # the psX weight-load (which waits on the ACT exp) would
            # otherwise head-of-line-block the next head's psD.
            def mk_psD(gh):
                lc_, h_ = divmod(gh, H)
                qth = qt_all[(h_ % 2) * DH : (h_ % 2 + 1) * DH, lc_, h_ // 2, :]
                kph = kpp[h_ // 2][(h_ % 2) * DH : (h_ % 2 + 1) * DH, :]
                psD = psA.tile([P, LCH], F32, tag="big")
                nc.tensor.matmul(psD[:], kph, qth, start=True, stop=True)
                return psD

            NGH = NLC * H
            psD_q = [mk_psD(0), mk_psD(1)]
            for lc in range(NLC):
                ot = outpool.tile([P, LT4, J], F32, tag="ot")
                for h in range(H):
                    gh = lc * H + h
                    psD = psD_q.pop(0)
                    ex = exppool.tile([P, LCH], BF16, tag="ex")
                    nc.scalar.activation(
                        ex[:], psD[:], mybir.ActivationFunctionType.Exp
                    )
                    if gh + 2 < NGH:
                        psD_q.append(mk_psD(gh + 2))
                    psX = psB.tile([P, LT4, DH + 1], F32, tag="small")
                    for lt in range(LT4):
                        nc.tensor.matmul(
                            psX[:, lt, :], ex[:, lt * P : (lt + 1) * P], vpa[h][:],
                            start=True, stop=True,
                        )
                    rc = recpool.tile([P, LT4], F32, tag="rc")
                    nc.vector.reciprocal(rc[:], psX[:, :, DH])
                    nc.any.tensor_tensor(
                        ot[:, :, h * DH : (h + 1) * DH],
                        psX[:, :, 0:DH],
                        rc[:].to_broadcast([P, LT4, DH]),
                        mult,
                    )
                nc.sync.dma_start(
                    outr[:, lc * LT4 : (lc + 1) * LT4, :], ot[:]
                )

    return nc


def _get_program():
    global _PROGRAM
    if _PROGRAM is None:
        _PROGRAM = _build_program()
    return _PROGRAM


def kernel(x, Wq, bq, Wk, bk, Wv, bv, E):
    global LAST_RESULTS
    x = np.ascontiguousarray(np.asarray(x, dtype=np.float32))
    Wq = np.asarray(Wq, dtype=np.float32)
    bq = np.asarray(bq, dtype=np.float32)
    Wk = np.asarray(Wk, dtype=np.float32)
    bk = np.asarray(bk, dtype=np.float32)
    Wv = np.asarray(Wv, dtype=np.float32)
    bv = np.asarray(bv, dtype=np.float32)
    E = np.asarray(E, dtype=np.float32)

    bf16 = mybir.dt.np(BF16)
    scale = 1.0 / math.sqrt(DH)
    xTs = [np.ascontiguousarray(x[b].T.astype(bf16)) for b in range(B)]
    in_maps = []
    for core in range(NCORES):
        b = core % B
        hg = core // B
        js = slice(hg * J, (hg + 1) * J)
        hs = slice(hg * H, (hg + 1) * H)
        wqTs = np.ascontiguousarray((Wq[js, :] * scale).T.astype(bf16))
        wkTs = np.ascontiguousarray(Wk[js, :].T.astype(bf16))
        wvTs = np.ascontiguousarray(Wv[js, :].T.astype(bf16))
        bqTs = np.ascontiguousarray((bq[js] * scale).reshape(JT, P).T)
        bkBs = np.ascontiguousarray(np.broadcast_to(bk[js], (P, J)))
        bvBs = np.ascontiguousarray(np.broadcast_to(bv[js], (P, J)))
        E_s = E[hs]  # [H, KK, L]
        eTs = np.ascontiguousarray(
            E_s.reshape(H, KK, NLC, LT4, P).transpose(2, 4, 0, 3, 1).astype(bf16)
        )  # [NLC, P, H, LT4, KK] bf16
        in_maps.append(
            {
                "xT": xTs[b],
                "wqT": wqTs,
                "wkT": wkTs,
                "wvT": wvTs,
                "bqT": bqTs,
                "bkB": bkBs,
                "bvB": bvBs,
                "eT": eTs,
            }
        )

    nc = _get_program()
    res = run_bass_kernel_spmd(nc, in_maps, list(range(NCORES)), trace=TRACE)
    LAST_RESULTS = res

    outp = np.empty((B, L, D), dtype=np.float32)
    for core in range(NCORES):
        b = core % B
        hg = core // B
        outp[b, :, hg * J : (hg + 1) * J] = res.results[core]["out"]
    return outp


# revision 15
# speedup vs baseline: 1.6767x; 1.0003x over previous
"""Linformer attention TRN2 Bass kernel (v2).

Problem: nn_LinformerAttention (B=4, L=4096, D=1024, NH=16, DH=64, k=128).

Sharding: 8 cores = batch(4) x head-group(2). Core c handles batch c%4 and
heads (c//4)*8 .. +8, producing out[b, :, hg*512:(hg+1)*512]. Slices are
disjoint -> no collectives; host reassembles.

Device algorithm per core:
  phase 1, streamed over 8 l-chunks of 512:
    - K = x @ Wk.T + bk, V likewise (fp32r matmuls, PSUM accum over 8
      d-subtiles of 128; bias added on DVE, K/V tiles written as bf16)
    - Q.T = Wq @ x.T + bq (scaled by 1/sqrt(dh) folded into Wq/bq on host),
      written bf16 and kept RESIDENT in SBUF for phase 2 (no DRAM spill)
    - KVp[h] += E_h-chunk @ [K_h | V_h]  (bf16 matmuls, full-rate;
      accumulated into SBUF fp32 via DVE adds)
  phase 2:
    - KpT[h] = PE-transpose(Kp[h]) -> bf16; Vp_aug[h] = [Vp[h] | ones] bf16
    - dotT[k, l] = KpT.T @ Q.T-chunk  (bf16; Q head slice read at partition
      offset (h%2)*64 straight out of the resident qt tile)
    - expT = exp(dotT) -> bf16       (ACT; logits are small by construction,
                                      exp without max-subtraction is safe)
    - psX[lt] = expT-tile.T @ Vp_aug -> [4, l-tile, 65] in ONE PSUM tile;
      col 64 = softmax denominator
    - one strided reciprocal + one fused broadcast-multiply normalizes all
      4 l-tiles of a head at once
  DMA: x-chunk0 + Wk emitted before the other weights so the first matmul
  starts ~12us in instead of waiting for the whole constant batch.

Host prep (numpy, outside HW-timed region): x[b].T, W slices pre-transposed
(+1/8 scale on Wq), E head-slices pre-transposed and cast to bf16, bias
tiles.
"""

import sys

sys.path.insert(0, "/opt/trn_rl_repo")

import math
from contextlib import ExitStack

import numpy as np

import json

import concourse.bass as bass
import concourse.bass2jax as bass2jax
import concourse.mybir as mybir
import concourse.tile as tile
from concourse.bass_utils import compile_bir_kernel as _orig_compile_bir_kernel
from concourse.bass_utils import run_bass_kernel_spmd
from concourse.masks import make_identity


def _split_multiwaits(bir_json_bytes):
    """This container's walrus encodes at most ONE sync wait per engine
    instruction ("Too many sync wait commands" otherwise), while Tile emits
    multi-wait instructions. Hoist extra waits onto single-wait
    EventSemaphore carrier instructions placed just before, on the same
    engine queue — semantically identical stalling."""
    bj = json.loads(bir_json_bytes)
    for fn in bj["functions"]:
        for blk in fn["blocks"]:
            out = []
            for inst in blk["instructions"]:
                si = inst.get("sync_info")
                waits = (si or {}).get("on_wait") or []
                if si and len(waits) > 1:
                    for wi, w in enumerate(waits[:-1]):
                        out.append(
                            {
                                "debug": inst.get("debug", 0),
                                "engine": inst.get("engine"),
                                "ins": [],
                                "outs": [],
                                "name": inst["name"] + "-w%d" % wi,
                                "opcode": "EventSemaphore",
                                "sync_info": {"on_update": [], "on_wait": [w]},
                            }
                        )
                    si["on_wait"] = [waits[-1]]
                out.append(inst)
            blk["instructions"] = out
    return json.dumps(bj).encode()


def _patched_compile_bir_kernel(bir_json, tmpdir, neff_name="file.neff"):
    return _orig_compile_bir_kernel(_split_multiwaits(bir_json), tmpdir, neff_name)


bass2jax.compile_bir_kernel = _patched_compile_bir_kernel

B, L, D = 4, 4096, 1024
NH, DH, KK = 16, 64, 128
NCORES = 8
HGS = 2  # head groups
H = NH // HGS  # 8 local heads per core
J = H * DH  # 512 output columns per core
P = 128
LCH = 512  # l-chunk
NLC = L // LCH  # 8
DC = D // P  # 8 contraction subtiles
JT = J // P  # 4
LT4 = LCH // P  # 4 l-tiles per chunk
F32 = mybir.dt.float32
F32R = mybir.dt.float32r  # full-rate PE matmul, TF32-like product precision
BF16 = mybir.dt.bfloat16

TRACE = False  # test.py sets True to collect a profile
LAST_RESULTS = None  # BassKernelResults of the last kernel() call

_PROGRAM = None


def _build_program():
    nc = bass.Bass()
    xT = nc.declare_dram_parameter("xT", [D, L], BF16, isOutput=False)
    wqT = nc.declare_dram_parameter("wqT", [D, J], BF16, isOutput=False)
    wkT = nc.declare_dram_parameter("wkT", [D, J], BF16, isOutput=False)
    wvT = nc.declare_dram_parameter("wvT", [D, J], BF16, isOutput=False)
    bqT = nc.declare_dram_parameter("bqT", [P, JT], F32, isOutput=False)
    bkB = nc.declare_dram_parameter("bkB", [P, J], F32, isOutput=False)
    bvB = nc.declare_dram_parameter("bvB", [P, J], F32, isOutput=False)
    eT = nc.declare_dram_parameter("eT", [NLC, P, H, LT4, KK], BF16, isOutput=False)
    out = nc.declare_dram_parameter("out", [L, J], F32, isOutput=True)

    add = mybir.AluOpType.add
    mult = mybir.AluOpType.mult

    with tile.TileContext(nc) as tc:
        with ExitStack() as ctx:
            const = ctx.enter_context(tc.tile_pool(name="const", bufs=1))
            xpool = ctx.enter_context(tc.tile_pool(name="x", bufs=2))
            kvpool = ctx.enter_context(tc.tile_pool(name="kv", bufs=4))
            epool = ctx.enter_context(tc.tile_pool(name="e", bufs=2))
            exppool = ctx.enter_context(tc.tile_pool(name="ex", bufs=3))
            outpool = ctx.enter_context(tc.tile_pool(name="ot", bufs=2))
            recpool = ctx.enter_context(tc.tile_pool(name="rc", bufs=4))
            psA = ctx.enter_context(tc.tile_pool(name="psA", bufs=4, space="PSUM"))
            psB = ctx.enter_context(tc.tile_pool(name="psB", bufs=4, space="PSUM"))

            xTr = xT[:, :].rearrange("(po pi) l -> pi po l", pi=P)
            outr = out[:, :].rearrange("(lo li) j -> li lo j", li=P)

            # ---- first x chunk + Wk + Wv first: the DMA engines stripe the
            # emitted batch in order, so what the first matmuls need must
            # head the queue. Everything is bf16 so the critical prefix
            # (x0+wk+wv ~ 3.2 MB) lands in ~9 us.
            x_cur = xpool.tile([P, DC, LCH], BF16, tag="x")
            nc.sync.dma_start(x_cur[:], xTr[:, :, 0:LCH])
            wk_sb = const.tile([P, DC, J], BF16, tag="wk")
            nc.sync.dma_start(wk_sb[:], wkT[:, :].rearrange("(po pi) j -> pi po j", pi=P))
            wv_sb = const.tile([P, DC, J], BF16, tag="wv")
            nc.sync.dma_start(wv_sb[:], wvT[:, :].rearrange("(po pi) j -> pi po j", pi=P))
            wq_sb = const.tile([P, DC, J], BF16, tag="wq")
            nc.sync.dma_start(wq_sb[:], wqT[:, :].rearrange("(po pi) j -> pi po j", pi=P))
            e_cur = epool.tile([P, H, LT4, KK], BF16, tag="e")
            nc.sync.dma_start(e_cur[:], eT[0])
            bkB_sb = const.tile([P, J], F32, tag="bkB")
            bvB_sb = const.tile([P, J], F32, tag="bvB")
            bqT_sb = const.tile([P, JT], F32, tag="bqT")
            nc.sync.dma_start(bkB_sb[:], bkB[:, :])
            nc.sync.dma_start(bvB_sb[:], bvB[:, :])
            nc.sync.dma_start(bqT_sb[:], bqT[:, :])
            ident = const.tile([P, P], F32, tag="ident")
            make_identity(nc, ident[:])

            # Warm-up: make PE observe each weight DMA individually, so no
            # later matmul ever needs two DMA-queue waits at once (the PE
            # Matmult encoding only fits one sync wait -> neuronxcc
            # "Too many sync wait commands" otherwise). wq's warm-up is
            # deferred until just before the first Q matmul so the K/V
            # stream isn't gated on the wq DMA.
            def warm(w_sb, wi):
                ps_w = psB.tile([1, 1], F32, tag="small", name=f"warm{wi}")
                nc.tensor.matmul(
                    ps_w[:], w_sb[:, 0, 0:1].bitcast(BF16),
                    w_sb[:, 0, 0:1].bitcast(BF16),
                    start=True, stop=True,
                )

            warm(wk_sb, 0)

            # Per head PAIR (jt): [k, K|V, head-in-pair, dh] so that one PE
            # transpose of the K plane puts head 2jt's KpT on partitions
            # 0..63 and head 2jt+1's on 64..127 — matching the partition
            # offset of the resident Q slices (matmul requires equal base
            # partitions for stationary and moving operands).
            kvp_acc = [const.tile([P, 2, 2, DH], F32, tag=f"kvp{j}", name=f"kvp{j}") for j in range(JT)]
            kpp = [const.tile([P, KK], BF16, tag=f"kpp{j}", name=f"kpp{j}") for j in range(JT)]
            vpa = [const.tile([P, DH + 1], BF16, tag=f"vpa{h}", name=f"vpa{h}") for h in range(H)]
            # Q.T resident in SBUF, bf16: [dh|dh, lc, jt, l]; head h of chunk
            # lc lives at partition offset (h%2)*64, plane jt=h//2.
            qt_all = const.tile([P, NLC, JT, LCH], BF16, tag="qt")

            # ---- phase 1: projections + Linformer K/V reduction
            for lc in range(NLC):
                x_sb, e_sb = x_cur, e_cur
                if lc + 1 < NLC:
                    x_cur = xpool.tile([P, DC, LCH], BF16, tag="x")
                    nc.sync.dma_start(
                        x_cur[:], xTr[:, :, (lc + 1) * LCH : (lc + 2) * LCH]
                    )
                    e_cur = epool.tile([P, H, LT4, KK], BF16, tag="e")
                    nc.sync.dma_start(e_cur[:], eT[lc + 1])
                kv_tiles = []
                if lc == 0:
                    # Chunk 0: all-K sweep first (needs only x0+wk, the head
                    # of the DMA queue), V after its warm-up — the PE never
                    # stalls on the wv/wq DMAs this way.
                    kv_tiles = [
                        kvpool.tile([P, 2, LCH], BF16, tag="kv", name=f"kv0_{lt}")
                        for lt in range(LT4)
                    ]
                    psKs = []
                    for lt in range(LT4):
                        psK = psA.tile([P, LCH], F32, tag="big")
                        for dc in range(DC):
                            nc.tensor.matmul(
                                psK[:], x_sb[:, dc, lt * P : (lt + 1) * P],
                                wk_sb[:, dc, :],
                                start=(dc == 0), stop=(dc == DC - 1),
                            )
                        psKs.append(psK)
                    warm(wv_sb, 1)
                    for lt in range(LT4):
                        nc.any.tensor_tensor(
                            kv_tiles[lt][:, 0, :], psKs[lt][:], bkB_sb[:], add
                        )
                        psV = psA.tile([P, LCH], F32, tag="big")
                        for dc in range(DC):
                            nc.tensor.matmul(
                                psV[:], x_sb[:, dc, lt * P : (lt + 1) * P],
                                wv_sb[:, dc, :],
                                start=(dc == 0), stop=(dc == DC - 1),
                            )
                        nc.any.tensor_tensor(
                            kv_tiles[lt][:, 1, :], psV[:], bvB_sb[:], add
                        )
                else:
                    for lt in range(LT4):
                        psK = psA.tile([P, LCH], F32, tag="big")
                        psV = psA.tile([P, LCH], F32, tag="big")
                        for dc in range(DC):
                            xst = x_sb[:, dc, lt * P : (lt + 1) * P]
                            nc.tensor.matmul(
                                psK[:], xst,
                                wk_sb[:, dc, :],
                                start=(dc == 0), stop=(dc == DC - 1),
                            )
                            nc.tensor.matmul(
                                psV[:], xst,
                                wv_sb[:, dc, :],
                                start=(dc == 0), stop=(dc == DC - 1),
                            )
                        kv_sb = kvpool.tile([P, 2, LCH], BF16, tag="kv")
                        nc.any.tensor_tensor(kv_sb[:, 0, :], psK[:], bkB_sb[:], add)
                        nc.any.tensor_tensor(kv_sb[:, 1, :], psV[:], bvB_sb[:], add)
                        kv_tiles.append(kv_sb)
                if lc == 0:
                    warm(wq_sb, 2)
                for jt in range(JT):
                    psQ = psA.tile([P, LCH], F32, tag="big")
                    for dc in range(DC):
                        nc.tensor.matmul(
                            psQ[:], wq_sb[:, dc, jt * P : (jt + 1) * P],
                            x_sb[:, dc, :],
                            start=(dc == 0), stop=(dc == DC - 1),
                        )
                    nc.any.tensor_scalar(
                        qt_all[:, lc, jt, :], psQ[:], bqT_sb[:, jt : jt + 1], None, add
                    )
                for h in range(H):
                    psKV = psB.tile([P, 2, DH], F32, tag="small")
                    for lt in range(LT4):
                        nc.tensor.matmul(
                            psKV[:], e_sb[:, h, lt, :],
                            kv_tiles[lt][:, :, h * DH : (h + 1) * DH],
                            start=(lt == 0), stop=(lt == LT4 - 1),
                        )
                    acc = kvp_acc[h // 2][:, :, h % 2, :]
                    if lc == 0:
                        nc.any.tensor_copy(acc, psKV[:])
                    else:
                        nc.any.tensor_tensor(acc, acc, psKV[:], add)

            # ---- phase 2: attention
            for j in range(JT):
                psT = psB.tile([P, KK], F32, tag="small")
                nc.tensor.transpose(
                    psT[:], kvp_acc[j][:, 0, :, :], ident[:]
                )
                nc.any.tensor_copy(kpp[j][:], psT[:])
            for h in range(H):
                nc.any.tensor_copy(
                    vpa[h][:, 0:DH], kvp_acc[h // 2][:, 1, h % 2, :]
                )
                nc.any.memset(vpa[h][:, DH : DH + 1], 1.0)

            # psD runs two heads ahead of psX: the PE queue is in-order, so
            # the psX weight-load (which waits on the ACT exp) would
            # otherwise head-of-line-block the next head's psD.
            # psD emitted as 4 independent 128-row matmuls: short matmuls
            # pipeline at the PE issue floor (~55 ns) while a single 512-row
            # one pays ~200 ns of drain/fill against the neighboring psX
            # quads.
            def mk_psD(gh):
                lc_, h_ = divmod(gh, H)
                qth = qt_all[(h_ % 2) * DH : (h_ % 2 + 1) * DH, lc_, h_ // 2, :]
                kph = kpp[h_ // 2][(h_ % 2) * DH : (h_ % 2 + 1) * DH, :]
                psD = psA.tile([P, LCH], F32, tag="big")
                for lt in range(LT4):
                    nc.tensor.matmul(
                        psD[:, lt * P : (lt + 1) * P],
                        kph,
                        qth[:, lt * P : (lt + 1) * P],
                        start=True, stop=True,
                    )
                return psD

            NGH = NLC * H
            psD_q = [mk_psD(0), mk_psD(1)]
            for lc in range(NLC):
                ot = outpool.tile([P, LT4, J], F32, tag="ot")
                for h in range(H):
                    gh = lc * H + h
                    psD = psD_q.pop(0)
                    ex = exppool.tile([P, LCH], BF16, tag="ex")
                    nc.scalar.activation(
                        ex[:], psD[:], mybir.ActivationFunctionType.Exp
                    )
                    if gh + 2 < NGH:
                        psD_q.append(mk_psD(gh + 2))
                    psX = psB.tile([P, LT4, DH + 1], F32, tag="small")
                    for lt in range(LT4):
                        nc.tensor.matmul(
                            psX[:, lt, :], ex[:, lt * P : (lt + 1) * P], vpa[h][:],
                            start=True, stop=True,
                        )
                    rc = recpool.tile([P, LT4], F32, tag="rc")
                    nc.vector.reciprocal(rc[:], psX[:, :, DH])
                    nc.any.tensor_tensor(
                        ot[:, :, h * DH : (h + 1) * DH],
                        psX[:, :, 0:DH],
                        rc[:].to_broadcast([P, LT4, DH]),
                        mult,
                    )
                nc.sync.dma_start(
                    outr[:, lc * LT4 : (lc + 1) * LT4, :], ot[:]
                )

    return nc


def _get_program():
    global _PROGRAM
    if _PROGRAM is None:
        _PROGRAM = _build_program()
    return _PROGRAM


def kernel(x, Wq, bq, Wk, bk, Wv, bv, E):
    global LAST_RESULTS
    x = np.ascontiguousarray(np.asarray(x, dtype=np.float32))
    Wq = np.asarray(Wq, dtype=np.float32)
    bq = np.asarray(bq, dtype=np.float32)
    Wk = np.asarray(Wk, dtype=np.float32)
    bk = np.asarray(bk, dtype=np.float32)
    Wv = np.asarray(Wv, dtype=np.float32)
    bv = np.asarray(bv, dtype=np.float32)
    E = np.asarray(E, dtype=np.float32)

    bf16 = mybir.dt.np(BF16)
    scale = 1.0 / math.sqrt(DH)
    xTs = [np.ascontiguousarray(x[b].T.astype(bf16)) for b in range(B)]
    in_maps = []
    for core in range(NCORES):
        b = core % B
        hg = core // B
        js = slice(hg * J, (hg + 1) * J)
        hs = slice(hg * H, (hg + 1) * H)
        wqTs = np.ascontiguousarray((Wq[js, :] * scale).T.astype(bf16))
        wkTs = np.ascontiguousarray(Wk[js, :].T.astype(bf16))
        wvTs = np.ascontiguousarray(Wv[js, :].T.astype(bf16))
        bqTs = np.ascontiguousarray((bq[js] * scale).reshape(JT, P).T)
        bkBs = np.ascontiguousarray(np.broadcast_to(bk[js], (P, J)))
        bvBs = np.ascontiguousarray(np.broadcast_to(bv[js], (P, J)))
        E_s = E[hs]  # [H, KK, L]
        eTs = np.ascontiguousarray(
            E_s.reshape(H, KK, NLC, LT4, P).transpose(2, 4, 0, 3, 1).astype(bf16)
        )  # [NLC, P, H, LT4, KK] bf16
        in_maps.append(
            {
                "xT": xTs[b],
                "wqT": wqTs,
                "wkT": wkTs,
                "wvT": wvTs,
                "bqT": bqTs,
                "bkB": bkBs,
                "bvB": bvBs,
                "eT": eTs,
            }
        )

    nc = _get_program()
    res = run_bass_kernel_spmd(nc, in_maps, list(range(NCORES)), trace=TRACE)
    LAST_RESULTS = res

    outp = np.empty((B, L, D), dtype=np.float32)
    for core in range(NCORES):
        b = core % B
        hg = core // B
        outp[b, :, hg * J : (hg + 1) * J] = res.results[core]["out"]
    return outp


# revision 17
# speedup vs baseline: 1.6963x; 1.0117x over previous
"""Linformer attention TRN2 Bass kernel (v5).

Problem: nn_LinformerAttention (B=4, L=4096, D=1024, NH=16, DH=64, k=128).

Sharding: 8 cores = batch(4) x head-group(2). Core c handles batch c%4 and
heads (c//4)*8 .. +8, producing out[b, :, hg*512:(hg+1)*512]. Slices are
disjoint -> no collectives; host reassembles.

Device algorithm per core (bf16 inputs, fp32 PSUM accumulation):
  phase 1, streamed over 8 l-chunks of 512:
    - K = x @ Wk.T + bk, V likewise, K/V of one l-tile share a 2-bank PSUM
      pair tile; bias added on DVE, K/V written bf16
    - Q.T = Wq @ x.T + bq (1/sqrt(dh) folded into Wq/bq on host), bf16,
      RESIDENT in SBUF (no DRAM spill); head pairs are stacked on
      partitions (dh even head on 0..63, odd on 64..127)
    - KVp[h] += E_h-chunk @ [K_h | V_h]  (bf16, accumulated via DVE)
  phase 2 (head PAIRS, full 128-partition contraction):
    - the dh=64 contraction of a single head runs the PE at HALF rate, so
      heads are processed in pairs with a block-diagonal stationary
      kpd[j][kh] = diag(KpT_even[:, kh], KpT_odd[:, kh]) built once from a
      PE transpose; dot for both heads of chunk lc is TWO full-rate
      512-row matmuls into one 2-bank PSUM pair tile
    - ONE exp over the pair tile [128, 2, 512] (ACT amortized; logits are
      small by construction, exp without max-subtraction is safe)
    - psX accumulates the two k-halves: stationary = exp tile, moving =
      zero-padded paired [Vp | ones] (vpb) -> [l-tile, head, 65];
      col 64 = softmax denominator
    - one strided reciprocal + one fused 4-D broadcast-multiply per
      half-chunk normalizes both heads at once (alternating DVE/GpSimd)
    - psD is issued two pairs ahead (in-order PE queue would otherwise
      head-of-line block on the ACT exp)
  DMA: x-chunk0 + Wk head the queue (transfers complete in emission
  order at ~374 GB/s aggregate); wq's PE warm-up is deferred until the
  first Q matmul so K/V are not gated on the wq DMA.

Host prep (numpy, outside HW-timed region): x[b].T, W slices pre-transposed
(+1/8 scale on Wq), E head-slices pre-transposed, all cast bf16; bias
tiles fp32.
"""

import sys

sys.path.insert(0, "/opt/trn_rl_repo")

import math
from contextlib import ExitStack

import numpy as np

import json

import concourse.bass as bass
import concourse.bass2jax as bass2jax
import concourse.mybir as mybir
import concourse.tile as tile
from concourse.bass_utils import compile_bir_kernel as _orig_compile_bir_kernel
from concourse.bass_utils import run_bass_kernel_spmd
from concourse.masks import make_identity


def _split_multiwaits(bir_json_bytes):
    """This container's walrus encodes at most ONE sync wait per engine
    instruction ("Too many sync wait commands" otherwise), while Tile emits
    multi-wait instructions. Hoist extra waits onto single-wait
    EventSemaphore carrier instructions placed just before, on the same
    engine queue — semantically identical stalling."""
    bj = json.loads(bir_json_bytes)
    for fn in bj["functions"]:
        for blk in fn["blocks"]:
            out = []
            for inst in blk["instructions"]:
                si = inst.get("sync_info")
                waits = (si or {}).get("on_wait") or []
                if si and len(waits) > 1:
                    for wi, w in enumerate(waits[:-1]):
                        out.append(
                            {
                                "debug": inst.get("debug", 0),
                                "engine": inst.get("engine"),
                                "ins": [],
                                "outs": [],
                                "name": inst["name"] + "-w%d" % wi,
                                "opcode": "EventSemaphore",
                                "sync_info": {"on_update": [], "on_wait": [w]},
                            }
                        )
                    si["on_wait"] = [waits[-1]]
                out.append(inst)
            blk["instructions"] = out
    return json.dumps(bj).encode()


def _patched_compile_bir_kernel(bir_json, tmpdir, neff_name="file.neff"):
    return _orig_compile_bir_kernel(_split_multiwaits(bir_json), tmpdir, neff_name)


bass2jax.compile_bir_kernel = _patched_compile_bir_kernel

B, L, D = 4, 4096, 1024
NH, DH, KK = 16, 64, 128
NCORES = 8
HGS = 2  # head groups
H = NH // HGS  # 8 local heads per core
J = H * DH  # 512 output columns per core
P = 128
LCH = 512  # l-chunk
NLC = L // LCH  # 8
DC = D // P  # 8 contraction subtiles
JT = J // P  # 4 head pairs
LT4 = LCH // P  # 4 l-tiles per chunk
F32 = mybir.dt.float32
BF16 = mybir.dt.bfloat16

TRACE = False  # test.py sets True to collect a profile
LAST_RESULTS = None  # BassKernelResults of the last kernel() call

_PROGRAM = None


def _build_program():
    nc = bass.Bass()
    xT = nc.declare_dram_parameter("xT", [D, L], BF16, isOutput=False)
    wqT = nc.declare_dram_parameter("wqT", [D, J], BF16, isOutput=False)
    wkT = nc.declare_dram_parameter("wkT", [D, J], BF16, isOutput=False)
    wvT = nc.declare_dram_parameter("wvT", [D, J], BF16, isOutput=False)
    bqT = nc.declare_dram_parameter("bqT", [P, JT], F32, isOutput=False)
    bkB = nc.declare_dram_parameter("bkB", [P, J], F32, isOutput=False)
    bvB = nc.declare_dram_parameter("bvB", [P, J], F32, isOutput=False)
    eT = nc.declare_dram_parameter("eT", [NLC, P, H, LT4, KK], BF16, isOutput=False)
    out = nc.declare_dram_parameter("out", [L, J], F32, isOutput=True)

    add = mybir.AluOpType.add
    mult = mybir.AluOpType.mult

    with tile.TileContext(nc) as tc:
        with ExitStack() as ctx:
            const = ctx.enter_context(tc.tile_pool(name="const", bufs=1))
            xpool = ctx.enter_context(tc.tile_pool(name="x", bufs=2))
            kvpool = ctx.enter_context(tc.tile_pool(name="kv", bufs=4))
            epool = ctx.enter_context(tc.tile_pool(name="e", bufs=2))
            exppool = ctx.enter_context(tc.tile_pool(name="ex", bufs=3))
            outpool = ctx.enter_context(tc.tile_pool(name="ot", bufs=2))
            recpool = ctx.enter_context(tc.tile_pool(name="rc", bufs=4))
            # PSUM: 3 x 2-bank pair tiles + 2 x 1-bank small = 8 banks
            psA = ctx.enter_context(tc.tile_pool(name="psA", bufs=3, space="PSUM"))
            psB = ctx.enter_context(tc.tile_pool(name="psB", bufs=2, space="PSUM"))

            xTr = xT[:, :].rearrange("(po pi) l -> pi po l", pi=P)
            outr = out[:, :].rearrange("(lo li) j -> li lo j", li=P)

            # ---- DMA queue: what the first matmuls need heads the queue.
            x_cur = xpool.tile([P, DC, LCH], BF16, tag="x")
            nc.sync.dma_start(x_cur[:], xTr[:, :, 0:LCH])
            wk_sb = const.tile([P, DC, J], BF16, tag="wk")
            nc.sync.dma_start(wk_sb[:], wkT[:, :].rearrange("(po pi) j -> pi po j", pi=P))
            wv_sb = const.tile([P, DC, J], BF16, tag="wv")
            nc.sync.dma_start(wv_sb[:], wvT[:, :].rearrange("(po pi) j -> pi po j", pi=P))
            wq_sb = const.tile([P, DC, J], BF16, tag="wq")
            nc.sync.dma_start(wq_sb[:], wqT[:, :].rearrange("(po pi) j -> pi po j", pi=P))
            e_cur = epool.tile([P, H, LT4, KK], BF16, tag="e")
            nc.sync.dma_start(e_cur[:], eT[0])
            bkB_sb = const.tile([P, J], F32, tag="bkB")
            bvB_sb = const.tile([P, J], F32, tag="bvB")
            bqT_sb = const.tile([P, JT], F32, tag="bqT")
            nc.sync.dma_start(bkB_sb[:], bkB[:, :])
            nc.sync.dma_start(bvB_sb[:], bvB[:, :])
            nc.sync.dma_start(bqT_sb[:], bqT[:, :])
            ident = const.tile([P, P], F32, tag="ident")
            make_identity(nc, ident[:])
            identB = const.tile([P, P], BF16, tag="identB")
            make_identity(nc, identB[:])
            # Z[:, 0:128] = 0, Z[:, 128:256] = I: Z[:, 64+kh*64 :][p, m] is
            # the row-selector delta(p, m-64+kh*64) used to hoist the odd
            # head's Vp rows onto partitions 64..127.
            zsel = const.tile([P, 2 * P], BF16, tag="zsel")
            nc.any.memset(zsel[:, 0:P], 0.0)
            nc.any.tensor_copy(zsel[:, P : 2 * P], identB[:])

            # Warm-up: make PE observe each weight DMA individually, so no
            # later matmul ever needs two DMA-queue waits at once (the PE
            # Matmult encoding only fits one sync wait -> neuronxcc
            # "Too many sync wait commands" otherwise).
            def warm(w_sb, wi):
                ps_w = psB.tile([1, 1], F32, tag="small", name=f"warm{wi}")
                nc.tensor.matmul(
                    ps_w[:], w_sb[:, 0, 0:1],
                    w_sb[:, 0, 0:1],
                    start=True, stop=True,
                )

            warm(wk_sb, 0)

            # Per head PAIR j: [k, K|V, head-in-pair, dh]. One PE transpose
            # of the K plane puts head 2j's KpT rows on partitions 0..63 and
            # head 2j+1's on 64..127 — matching the resident Q layout.
            kvp_acc = [const.tile([P, 2, 2, DH], F32, tag=f"kvp{j}", name=f"kvp{j}") for j in range(JT)]
            # Block-diagonal stationaries diag(KpT_e[:,kh], KpT_o[:,kh])
            kpd = [
                [const.tile([P, KK], BF16, tag=f"kpd{j}_{kh}", name=f"kpd{j}_{kh}") for kh in range(2)]
                for j in range(JT)
            ]
            # Zero-padded paired [Vp | 1]: rows 0..63 = even head's k-half,
            # rows 64..127 = odd head's k-half, disjoint column blocks.
            vpa = [const.tile([P, DH + 1], BF16, tag=f"vpa{h}", name=f"vpa{h}") for h in range(H)]
            vpb = [
                [const.tile([P, 2, DH + 1], BF16, tag=f"vpb{j}_{kh}", name=f"vpb{j}_{kh}") for kh in range(2)]
                for j in range(JT)
            ]
            # Q.T resident in SBUF, bf16: [dh|dh, lc, j, l].
            qt_all = const.tile([P, NLC, JT, LCH], BF16, tag="qt")

            # ---- phase 1: projections + Linformer K/V reduction
            for lc in range(NLC):
                x_sb, e_sb = x_cur, e_cur
                if lc + 1 < NLC:
                    x_cur = xpool.tile([P, DC, LCH], BF16, tag="x")
                    nc.sync.dma_start(
                        x_cur[:], xTr[:, :, (lc + 1) * LCH : (lc + 2) * LCH]
                    )
                    e_cur = epool.tile([P, H, LT4, KK], BF16, tag="e")
                    nc.sync.dma_start(e_cur[:], eT[lc + 1])

                def mm_proj(ps_plane, lt, w_sb):
                    for dc in range(DC):
                        nc.tensor.matmul(
                            ps_plane, x_sb[:, dc, lt * P : (lt + 1) * P],
                            w_sb[:, dc, :],
                            start=(dc == 0), stop=(dc == DC - 1),
                        )

                kv_tiles = [None] * LT4
                ps_kv = [None] * LT4

                def finish_kv(lt):
                    kv_sb = kvpool.tile([P, 2, LCH], BF16, tag="kv", name=f"kv{lt}")
                    nc.any.tensor_tensor(kv_sb[:, 0, :], ps_kv[lt][:, 0, :], bkB_sb[:], add)
                    nc.any.tensor_tensor(kv_sb[:, 1, :], ps_kv[lt][:, 1, :], bvB_sb[:], add)
                    kv_tiles[lt] = kv_sb

                if lc == 0:
                    # Chunk 0: K sweeps first (needs only x0+wk, the head of
                    # the DMA queue); V after its warm-up so the PE never
                    # stalls on the wv/wq DMAs.
                    for lt in range(3):
                        ps_kv[lt] = psA.tile([P, 2, LCH], F32, tag="big", name=f"ps0_{lt}")
                        mm_proj(ps_kv[lt][:, 0, :], lt, wk_sb)
                    warm(wv_sb, 1)
                    mm_proj(ps_kv[0][:, 1, :], 0, wv_sb)
                    finish_kv(0)
                    mm_proj(ps_kv[1][:, 1, :], 1, wv_sb)
                    finish_kv(1)
                    ps_kv[3] = psA.tile([P, 2, LCH], F32, tag="big", name="ps0_3")
                    mm_proj(ps_kv[3][:, 0, :], 3, wk_sb)
                    mm_proj(ps_kv[2][:, 1, :], 2, wv_sb)
                    finish_kv(2)
                    mm_proj(ps_kv[3][:, 1, :], 3, wv_sb)
                    finish_kv(3)
                else:
                    for lt in range(LT4):
                        ps_kv[lt] = psA.tile([P, 2, LCH], F32, tag="big", name=f"ps_{lt}")
                        for dc in range(DC):
                            xst = x_sb[:, dc, lt * P : (lt + 1) * P]
                            nc.tensor.matmul(
                                ps_kv[lt][:, 0, :], xst,
                                wk_sb[:, dc, :],
                                start=(dc == 0), stop=(dc == DC - 1),
                            )
                            nc.tensor.matmul(
                                ps_kv[lt][:, 1, :], xst,
                                wv_sb[:, dc, :],
                                start=(dc == 0), stop=(dc == DC - 1),
                            )
                        finish_kv(lt)

                if lc == 0:
                    warm(wq_sb, 2)
                for jp in range(JT // 2):
                    psQ = psA.tile([P, 2, LCH], F32, tag="big", name="psQ")
                    for pl in range(2):
                        jt = jp * 2 + pl
                        for dc in range(DC):
                            nc.tensor.matmul(
                                psQ[:, pl, :], wq_sb[:, dc, jt * P : (jt + 1) * P],
                                x_sb[:, dc, :],
                                start=(dc == 0), stop=(dc == DC - 1),
                            )
                        nc.any.tensor_scalar(
                            qt_all[:, lc, jt, :], psQ[:, pl, :],
                            bqT_sb[:, jt : jt + 1], None, add,
                        )
                for h in range(H):
                    psKV = psB.tile([P, 2, DH], F32, tag="small")
                    for lt in range(LT4):
                        nc.tensor.matmul(
                            psKV[:], e_sb[:, h, lt, :],
                            kv_tiles[lt][:, :, h * DH : (h + 1) * DH],
                            start=(lt == 0), stop=(lt == LT4 - 1),
                        )
                    acc = kvp_acc[h // 2][:, :, h % 2, :]
                    if lc == 0:
                        nc.any.tensor_copy(acc, psKV[:])
                    else:
                        nc.any.tensor_tensor(acc, acc, psKV[:], add)

            # ---- phase 2 prep: block-diag KpT and zero-padded paired Vp
            for h in range(H):
                nc.any.tensor_copy(
                    vpa[h][:, 0:DH], kvp_acc[h // 2][:, 1, h % 2, :]
                )
                nc.any.memset(vpa[h][:, DH : DH + 1], 1.0)
            for j in range(JT):
                psT = psB.tile([P, KK], F32, tag="small")
                nc.tensor.transpose(
                    psT[:], kvp_acc[j][:, 0, :, :], ident[:]
                )
                for kh in range(2):
                    kp = kpd[j][kh]
                    nc.any.memset(kp[:], 0.0)
                    nc.any.tensor_copy(
                        kp[0:DH, 0:DH], psT[0:DH, kh * DH : (kh + 1) * DH]
                    )
                    nc.any.tensor_copy(
                        kp[DH:P, DH:P], psT[DH:P, kh * DH : (kh + 1) * DH]
                    )
                for kh in range(2):
                    psE = psB.tile([DH, DH + 1], F32, tag="small")
                    nc.tensor.matmul(
                        psE[:], identB[:, kh * DH : (kh + 1) * DH], vpa[2 * j][:],
                        start=True, stop=True,
                    )
                    psO = psB.tile([P, DH + 1], F32, tag="small")
                    nc.tensor.matmul(
                        psO[:], zsel[:, DH + kh * DH : DH + kh * DH + P],
                        vpa[2 * j + 1][:],
                        start=True, stop=True,
                    )
                    vb = vpb[j][kh]
                    nc.any.memset(vb[:], 0.0)
                    nc.any.tensor_copy(vb[0:DH, 0, :], psE[:])
                    nc.any.tensor_copy(vb[DH:P, 1, :], psO[DH:P, :])

            # ---- phase 2: attention over head pairs
            def mk_psDp(gp):
                lc_, j_ = divmod(gp, JT)
                psDp = psA.tile([P, 2, LCH], F32, tag="big", name="psDp")
                for kh in range(2):
                    nc.tensor.matmul(
                        psDp[:, kh, :], kpd[j_][kh][:], qt_all[:, lc_, j_, :],
                        start=True, stop=True,
                    )
                return psDp

            NGP = NLC * JT
            psD_q = [mk_psDp(0), mk_psDp(1)]
            for lc in range(NLC):
                ot = outpool.tile([P, LT4, JT, 2, DH], F32, tag="ot")
                for j in range(JT):
                    gp = lc * JT + j
                    psDp = psD_q.pop(0)
                    ex2 = exppool.tile([P, 2, LCH], BF16, tag="ex")
                    nc.scalar.activation(
                        ex2[:], psDp[:], mybir.ActivationFunctionType.Exp
                    )
                    if gp + 2 < NGP:
                        psD_q.append(mk_psDp(gp + 2))
                    for half in range(2):
                        psX = psB.tile([P, 2, 2, DH + 1], F32, tag="small")
                        for lti in range(2):
                            lt = half * 2 + lti
                            for kh in range(2):
                                nc.tensor.matmul(
                                    psX[:, lti, :, :],
                                    ex2[:, kh, lt * P : (lt + 1) * P],
                                    vpb[j][kh][:],
                                    start=(kh == 0), stop=(kh == 1),
                                )
                        rc = recpool.tile([P, 2, 2], F32, tag="rc")
                        nc.vector.reciprocal(rc[:], psX[:, :, :, DH])
                        nc.vector.tensor_tensor(
                            ot[:, half * 2 : half * 2 + 2, j, :, :],
                            psX[:, :, :, 0:DH],
                            rc[:].to_broadcast([P, 2, 2, DH]),
                            mult,
                        )
                nc.sync.dma_start(
                    outr[:, lc * LT4 : (lc + 1) * LT4, :], ot[:]
                )

    return nc


def _get_program():
    global _PROGRAM
    if _PROGRAM is None:
        _PROGRAM = _build_program()
    return _PROGRAM


def kernel(x, Wq, bq, Wk, bk, Wv, bv, E):
    global LAST_RESULTS
    x = np.ascontiguousarray(np.asarray(x, dtype=np.float32))
    Wq = np.asarray(Wq, dtype=np.float32)
    bq = np.asarray(bq, dtype=np.float32)
    Wk = np.asarray(Wk, dtype=np.float32)
    bk = np.asarray(bk, dtype=np.float32)
    Wv = np.asarray(Wv, dtype=np.float32)
    bv = np.asarray(bv, dtype=np.float32)
    E = np.asarray(E, dtype=np.float32)

    bf16 = mybir.dt.np(BF16)
    scale = 1.0 / math.sqrt(DH)
    xTs = [np.ascontiguousarray(x[b].T.astype(bf16)) for b in range(B)]
    in_maps = []
    for core in range(NCORES):
        b = core % B
        hg = core // B
        js = slice(hg * J, (hg + 1) * J)
        hs = slice(hg * H, (hg + 1) * H)
        wqTs = np.ascontiguousarray((Wq[js, :] * scale).T.astype(bf16))
        wkTs = np.ascontiguousarray(Wk[js, :].T.astype(bf16))
        wvTs = np.ascontiguousarray(Wv[js, :].T.astype(bf16))
        bqTs = np.ascontiguousarray((bq[js] * scale).reshape(JT, P).T)
        bkBs = np.ascontiguousarray(np.broadcast_to(bk[js], (P, J)))
        bvBs = np.ascontiguousarray(np.broadcast_to(bv[js], (P, J)))
        E_s = E[hs]  # [H, KK, L]
        eTs = np.ascontiguousarray(
            E_s.reshape(H, KK, NLC, LT4, P).transpose(2, 4, 0, 3, 1).astype(bf16)
        )  # [NLC, P, H, LT4, KK] bf16
        in_maps.append(
            {
                "xT": xTs[b],
                "wqT": wqTs,
                "wkT": wkTs,
                "wvT": wvTs,
                "bqT": bqTs,
                "bkB": bkBs,
                "bvB": bvBs,
                "eT": eTs,
            }
        )

    nc = _get_program()
    res = run_bass_kernel_spmd(nc, in_maps, list(range(NCORES)), trace=TRACE)
    LAST_RESULTS = res

    outp = np.empty((B, L, D), dtype=np.float32)
    for core in range(NCORES):
        b = core % B
        hg = core // B
        outp[b, :, hg * J : (hg + 1) * J] = res.results[core]["out"]
    return outp


# revision 18
# speedup vs baseline: 1.7151x; 1.0111x over previous
"""Linformer attention TRN2 Bass kernel (v5).

Problem: nn_LinformerAttention (B=4, L=4096, D=1024, NH=16, DH=64, k=128).

Sharding: 8 cores = batch(4) x head-group(2). Core c handles batch c%4 and
heads (c//4)*8 .. +8, producing out[b, :, hg*512:(hg+1)*512]. Slices are
disjoint -> no collectives; host reassembles.

Device algorithm per core (bf16 inputs, fp32 PSUM accumulation):
  phase 1, streamed over 8 l-chunks of 512:
    - K = x @ Wk.T + bk, V likewise, K/V of one l-tile share a 2-bank PSUM
      pair tile; bias added on DVE, K/V written bf16
    - Q.T = Wq @ x.T + bq (1/sqrt(dh) folded into Wq/bq on host), bf16,
      RESIDENT in SBUF (no DRAM spill); head pairs are stacked on
      partitions (dh even head on 0..63, odd on 64..127)
    - KVp[h] += E_h-chunk @ [K_h | V_h]  (bf16, accumulated via DVE)
  phase 2 (head PAIRS, full 128-partition contraction):
    - the dh=64 contraction of a single head runs the PE at HALF rate, so
      heads are processed in pairs with a block-diagonal stationary
      kpd[j][kh] = diag(KpT_even[:, kh], KpT_odd[:, kh]) built once from a
      PE transpose; dot for both heads of chunk lc is TWO full-rate
      512-row matmuls into one 2-bank PSUM pair tile
    - ONE exp over the pair tile [128, 2, 512] (ACT amortized; logits are
      small by construction, exp without max-subtraction is safe)
    - psX accumulates the two k-halves: stationary = exp tile, moving =
      zero-padded paired [Vp | ones] (vpb) -> [l-tile, head, 65];
      col 64 = softmax denominator
    - one strided reciprocal + one fused 4-D broadcast-multiply per
      half-chunk normalizes both heads at once (alternating DVE/GpSimd)
    - psD is issued two pairs ahead (in-order PE queue would otherwise
      head-of-line block on the ACT exp)
  DMA: x-chunk0 + Wk head the queue (transfers complete in emission
  order at ~374 GB/s aggregate); wq's PE warm-up is deferred until the
  first Q matmul so K/V are not gated on the wq DMA.

Host prep (numpy, outside HW-timed region): x[b].T, W slices pre-transposed
(+1/8 scale on Wq), E head-slices pre-transposed, all cast bf16; bias
tiles fp32.
"""

import sys

sys.path.insert(0, "/opt/trn_rl_repo")

import math
from contextlib import ExitStack

import numpy as np

import json

import concourse.bass as bass
import concourse.bass2jax as bass2jax
import concourse.mybir as mybir
import concourse.tile as tile
from concourse.bass_utils import compile_bir_kernel as _orig_compile_bir_kernel
from concourse.bass_utils import run_bass_kernel_spmd
from concourse.masks import make_identity


def _split_multiwaits(bir_json_bytes):
    """This container's walrus encodes at most ONE sync wait per engine
    instruction ("Too many sync wait commands" otherwise), while Tile emits
    multi-wait instructions. Hoist extra waits onto single-wait
    EventSemaphore carrier instructions placed just before, on the same
    engine queue — semantically identical stalling."""
    bj = json.loads(bir_json_bytes)
    for fn in bj["functions"]:
        for blk in fn["blocks"]:
            out = []
            for inst in blk["instructions"]:
                si = inst.get("sync_info")
                waits = (si or {}).get("on_wait") or []
                if si and len(waits) > 1:
                    for wi, w in enumerate(waits[:-1]):
                        out.append(
                            {
                                "debug": inst.get("debug", 0),
                                "engine": inst.get("engine"),
                                "ins": [],
                                "outs": [],
                                "name": inst["name"] + "-w%d" % wi,
                                "opcode": "EventSemaphore",
                                "sync_info": {"on_update": [], "on_wait": [w]},
                            }
                        )
                    si["on_wait"] = [waits[-1]]
                out.append(inst)
            blk["instructions"] = out
    return json.dumps(bj).encode()


def _patched_compile_bir_kernel(bir_json, tmpdir, neff_name="file.neff"):
    return _orig_compile_bir_kernel(_split_multiwaits(bir_json), tmpdir, neff_name)


bass2jax.compile_bir_kernel = _patched_compile_bir_kernel

B, L, D = 4, 4096, 1024
NH, DH, KK = 16, 64, 128
NCORES = 8
HGS = 2  # head groups
H = NH // HGS  # 8 local heads per core
J = H * DH  # 512 output columns per core
P = 128
LCH = 512  # l-chunk
NLC = L // LCH  # 8
DC = D // P  # 8 contraction subtiles
JT = J // P  # 4 head pairs
LT4 = LCH // P  # 4 l-tiles per chunk
F32 = mybir.dt.float32
BF16 = mybir.dt.bfloat16

TRACE = False  # test.py sets True to collect a profile
LAST_RESULTS = None  # BassKernelResults of the last kernel() call

_PROGRAM = None


def _build_program():
    nc = bass.Bass()
    xT = nc.declare_dram_parameter("xT", [D, L], BF16, isOutput=False)
    wqT = nc.declare_dram_parameter("wqT", [D, J], BF16, isOutput=False)
    wkT = nc.declare_dram_parameter("wkT", [D, J], BF16, isOutput=False)
    wvT = nc.declare_dram_parameter("wvT", [D, J], BF16, isOutput=False)
    bqT = nc.declare_dram_parameter("bqT", [P, JT], F32, isOutput=False)
    bkB = nc.declare_dram_parameter("bkB", [P, J], F32, isOutput=False)
    bvB = nc.declare_dram_parameter("bvB", [P, J], F32, isOutput=False)
    eT = nc.declare_dram_parameter("eT", [NLC, P, H, LT4, KK], BF16, isOutput=False)
    out = nc.declare_dram_parameter("out", [L, J], F32, isOutput=True)

    add = mybir.AluOpType.add
    mult = mybir.AluOpType.mult

    with tile.TileContext(nc) as tc:
        with ExitStack() as ctx:
            const = ctx.enter_context(tc.tile_pool(name="const", bufs=1))
            xpool = ctx.enter_context(tc.tile_pool(name="x", bufs=2))
            kvpool = ctx.enter_context(tc.tile_pool(name="kv", bufs=4))
            epool = ctx.enter_context(tc.tile_pool(name="e", bufs=2))
            exppool = ctx.enter_context(tc.tile_pool(name="ex", bufs=3))
            outpool = ctx.enter_context(tc.tile_pool(name="ot", bufs=2))
            recpool = ctx.enter_context(tc.tile_pool(name="rc", bufs=4))
            # PSUM: 3 x 2-bank pair tiles + 2 x 1-bank small = 8 banks
            psA = ctx.enter_context(tc.tile_pool(name="psA", bufs=3, space="PSUM"))
            psB = ctx.enter_context(tc.tile_pool(name="psB", bufs=2, space="PSUM"))

            xTr = xT[:, :].rearrange("(po pi) l -> pi po l", pi=P)
            outr = out[:, :].rearrange("(lo li) j -> li lo j", li=P)

            # ---- DMA queue: what the first matmuls need heads the queue.
            x_cur = xpool.tile([P, DC, LCH], BF16, tag="x")
            nc.sync.dma_start(x_cur[:], xTr[:, :, 0:LCH])
            wk_sb = const.tile([P, DC, J], BF16, tag="wk")
            nc.sync.dma_start(wk_sb[:], wkT[:, :].rearrange("(po pi) j -> pi po j", pi=P))
            wv_sb = const.tile([P, DC, J], BF16, tag="wv")
            nc.sync.dma_start(wv_sb[:], wvT[:, :].rearrange("(po pi) j -> pi po j", pi=P))
            wq_sb = const.tile([P, DC, J], BF16, tag="wq")
            nc.sync.dma_start(wq_sb[:], wqT[:, :].rearrange("(po pi) j -> pi po j", pi=P))
            e_cur = epool.tile([P, H, LT4, KK], BF16, tag="e")
            nc.sync.dma_start(e_cur[:], eT[0])
            bkB_sb = const.tile([P, J], F32, tag="bkB")
            bvB_sb = const.tile([P, J], F32, tag="bvB")
            bqT_sb = const.tile([P, JT], F32, tag="bqT")
            nc.sync.dma_start(bkB_sb[:], bkB[:, :])
            nc.sync.dma_start(bvB_sb[:], bvB[:, :])
            nc.sync.dma_start(bqT_sb[:], bqT[:, :])
            ident = const.tile([P, P], F32, tag="ident")
            make_identity(nc, ident[:])
            identB = const.tile([P, P], BF16, tag="identB")
            make_identity(nc, identB[:])
            # Z[:, 0:128] = 0, Z[:, 128:256] = I: Z[:, 64+kh*64 :][p, m] is
            # the row-selector delta(p, m-64+kh*64) used to hoist the odd
            # head's Vp rows onto partitions 64..127.
            zsel = const.tile([P, 2 * P], BF16, tag="zsel")
            nc.any.memset(zsel[:, 0:P], 0.0)
            nc.any.tensor_copy(zsel[:, P : 2 * P], identB[:])

            # Warm-up: make PE observe each weight DMA individually, so no
            # later matmul ever needs two DMA-queue waits at once (the PE
            # Matmult encoding only fits one sync wait -> neuronxcc
            # "Too many sync wait commands" otherwise).
            def warm(w_sb, wi):
                ps_w = psB.tile([1, 1], F32, tag="small", name=f"warm{wi}")
                nc.tensor.matmul(
                    ps_w[:], w_sb[:, 0, 0:1],
                    w_sb[:, 0, 0:1],
                    start=True, stop=True,
                )

            warm(wk_sb, 0)

            # Per head PAIR j: [k, K|V, head-in-pair, dh]. One PE transpose
            # of the K plane puts head 2j's KpT rows on partitions 0..63 and
            # head 2j+1's on 64..127 — matching the resident Q layout.
            kvp_acc = [const.tile([P, 2, 2, DH], F32, tag=f"kvp{j}", name=f"kvp{j}") for j in range(JT)]
            # Block-diagonal stationaries diag(KpT_e[:,kh], KpT_o[:,kh])
            kpd = [
                [const.tile([P, KK], BF16, tag=f"kpd{j}_{kh}", name=f"kpd{j}_{kh}") for kh in range(2)]
                for j in range(JT)
            ]
            # Zero-padded paired [Vp | 1]: rows 0..63 = even head's k-half,
            # rows 64..127 = odd head's k-half, disjoint column blocks.
            vpa = [const.tile([P, DH + 1], BF16, tag=f"vpa{h}", name=f"vpa{h}") for h in range(H)]
            vpb = [
                [const.tile([P, 2, DH + 1], BF16, tag=f"vpb{j}_{kh}", name=f"vpb{j}_{kh}") for kh in range(2)]
                for j in range(JT)
            ]
            # Q.T resident in SBUF, bf16: [dh|dh, lc, j, l].
            qt_all = const.tile([P, NLC, JT, LCH], BF16, tag="qt")

            # ---- phase 1: projections + Linformer K/V reduction
            for lc in range(NLC):
                x_sb, e_sb = x_cur, e_cur
                if lc + 1 < NLC:
                    x_cur = xpool.tile([P, DC, LCH], BF16, tag="x")
                    nc.sync.dma_start(
                        x_cur[:], xTr[:, :, (lc + 1) * LCH : (lc + 2) * LCH]
                    )
                    e_cur = epool.tile([P, H, LT4, KK], BF16, tag="e")
                    nc.sync.dma_start(e_cur[:], eT[lc + 1])

                def mm_proj(ps_plane, lt, w_sb):
                    for dc in range(DC):
                        nc.tensor.matmul(
                            ps_plane, x_sb[:, dc, lt * P : (lt + 1) * P],
                            w_sb[:, dc, :],
                            start=(dc == 0), stop=(dc == DC - 1),
                        )

                kv_tiles = [None] * LT4
                ps_kv = [None] * LT4

                def finish_kv(lt):
                    kv_sb = kvpool.tile([P, 2, LCH], BF16, tag="kv", name=f"kv{lt}")
                    nc.any.tensor_tensor(kv_sb[:, 0, :], ps_kv[lt][:, 0, :], bkB_sb[:], add)
                    nc.any.tensor_tensor(kv_sb[:, 1, :], ps_kv[lt][:, 1, :], bvB_sb[:], add)
                    kv_tiles[lt] = kv_sb

                if lc == 0:
                    # Chunk 0: K sweeps first (needs only x0+wk, the head of
                    # the DMA queue); V after its warm-up so the PE never
                    # stalls on the wv/wq DMAs.
                    for lt in range(3):
                        ps_kv[lt] = psA.tile([P, 2, LCH], F32, tag="big", name=f"ps0_{lt}")
                        mm_proj(ps_kv[lt][:, 0, :], lt, wk_sb)
                    warm(wv_sb, 1)
                    mm_proj(ps_kv[0][:, 1, :], 0, wv_sb)
                    finish_kv(0)
                    mm_proj(ps_kv[1][:, 1, :], 1, wv_sb)
                    finish_kv(1)
                    ps_kv[3] = psA.tile([P, 2, LCH], F32, tag="big", name="ps0_3")
                    mm_proj(ps_kv[3][:, 0, :], 3, wk_sb)
                    mm_proj(ps_kv[2][:, 1, :], 2, wv_sb)
                    finish_kv(2)
                    mm_proj(ps_kv[3][:, 1, :], 3, wv_sb)
                    finish_kv(3)
                else:
                    for lt in range(LT4):
                        ps_kv[lt] = psA.tile([P, 2, LCH], F32, tag="big", name=f"ps_{lt}")
                        for dc in range(DC):
                            xst = x_sb[:, dc, lt * P : (lt + 1) * P]
                            nc.tensor.matmul(
                                ps_kv[lt][:, 0, :], xst,
                                wk_sb[:, dc, :],
                                start=(dc == 0), stop=(dc == DC - 1),
                            )
                            nc.tensor.matmul(
                                ps_kv[lt][:, 1, :], xst,
                                wv_sb[:, dc, :],
                                start=(dc == 0), stop=(dc == DC - 1),
                            )
                        finish_kv(lt)

                if lc == 0:
                    warm(wq_sb, 2)
                for jp in range(JT // 2):
                    psQ = psA.tile([P, 2, LCH], F32, tag="big", name="psQ")
                    for pl in range(2):
                        jt = jp * 2 + pl
                        for dc in range(DC):
                            nc.tensor.matmul(
                                psQ[:, pl, :], wq_sb[:, dc, jt * P : (jt + 1) * P],
                                x_sb[:, dc, :],
                                start=(dc == 0), stop=(dc == DC - 1),
                            )
                        nc.any.tensor_scalar(
                            qt_all[:, lc, jt, :], psQ[:, pl, :],
                            bqT_sb[:, jt : jt + 1], None, add,
                        )
                for h in range(H):
                    psKV = psB.tile([P, 2, DH], F32, tag="small")
                    for lt in range(LT4):
                        nc.tensor.matmul(
                            psKV[:], e_sb[:, h, lt, :],
                            kv_tiles[lt][:, :, h * DH : (h + 1) * DH],
                            start=(lt == 0), stop=(lt == LT4 - 1),
                        )
                    acc = kvp_acc[h // 2][:, :, h % 2, :]
                    if lc == 0:
                        nc.any.tensor_copy(acc, psKV[:])
                    else:
                        nc.any.tensor_tensor(acc, acc, psKV[:], add)

            # ---- phase 2 prep: block-diag KpT and zero-padded paired Vp
            for h in range(H):
                nc.any.tensor_copy(
                    vpa[h][:, 0:DH], kvp_acc[h // 2][:, 1, h % 2, :]
                )
                nc.any.memset(vpa[h][:, DH : DH + 1], 1.0)
            for j in range(JT):
                psT = psB.tile([P, KK], F32, tag="small")
                nc.tensor.transpose(
                    psT[:], kvp_acc[j][:, 0, :, :], ident[:]
                )
                for kh in range(2):
                    kp = kpd[j][kh]
                    nc.any.memset(kp[:], 0.0)
                    nc.any.tensor_copy(
                        kp[0:DH, 0:DH], psT[0:DH, kh * DH : (kh + 1) * DH]
                    )
                    nc.any.tensor_copy(
                        kp[DH:P, DH:P], psT[DH:P, kh * DH : (kh + 1) * DH]
                    )
                for kh in range(2):
                    psE = psB.tile([DH, DH + 1], F32, tag="small")
                    nc.tensor.matmul(
                        psE[:], identB[:, kh * DH : (kh + 1) * DH], vpa[2 * j][:],
                        start=True, stop=True,
                    )
                    psO = psB.tile([P, DH + 1], F32, tag="small")
                    nc.tensor.matmul(
                        psO[:], zsel[:, DH + kh * DH : DH + kh * DH + P],
                        vpa[2 * j + 1][:],
                        start=True, stop=True,
                    )
                    vb = vpb[j][kh]
                    nc.any.memset(vb[:], 0.0)
                    nc.any.tensor_copy(vb[0:DH, 0, :], psE[:])
                    nc.any.tensor_copy(vb[DH:P, 1, :], psO[DH:P, :])

            # ---- phase 2: attention over head pairs
            def mk_psDp(gp):
                lc_, j_ = divmod(gp, JT)
                psDp = psA.tile([P, 2, LCH], F32, tag="big", name="psDp")
                for kh in range(2):
                    nc.tensor.matmul(
                        psDp[:, kh, :], kpd[j_][kh][:], qt_all[:, lc_, j_, :],
                        start=True, stop=True,
                    )
                return psDp

            NGP = NLC * JT
            psD_q = [mk_psDp(0)]
            for lc in range(NLC):
                ot = outpool.tile([P, LT4, JT, 2, DH], F32, tag="ot")
                for j in range(JT):
                    gp = lc * JT + j
                    psDp = psD_q.pop(0)
                    ex2 = exppool.tile([P, 2, LCH], BF16, tag="ex")
                    nc.scalar.activation(
                        ex2[:], psDp[:], mybir.ActivationFunctionType.Exp
                    )
                    if gp + 1 < NGP:
                        psD_q.append(mk_psDp(gp + 1))
                    # One 2-bank PSUM tile for all 4 l-tiles of the pair:
                    # 1024B lt-blocks keep each matmul dst inside one bank;
                    # a single fused broadcast-multiply then normalizes the
                    # whole pair (Vector was the co-bottleneck).
                    psX = psA.tile([P, LT4, 2, P], F32, tag="big", name="psX")
                    for lt in range(LT4):
                        for kh in range(2):
                            nc.tensor.matmul(
                                psX[:, lt, :, 0 : DH + 1],
                                ex2[:, kh, lt * P : (lt + 1) * P],
                                vpb[j][kh][:],
                                start=(kh == 0), stop=(kh == 1),
                            )
                    rc = recpool.tile([P, LT4, 2], F32, tag="rc")
                    nc.vector.reciprocal(rc[:, :, 0], psX[:, :, 0, DH])
                    nc.vector.reciprocal(rc[:, :, 1], psX[:, :, 1, DH])
                    nc.vector.tensor_tensor(
                        ot[:, :, j, :, :],
                        psX[:, :, :, 0:DH],
                        rc[:].to_broadcast([P, LT4, 2, DH]),
                        mult,
                    )
                nc.sync.dma_start(
                    outr[:, lc * LT4 : (lc + 1) * LT4, :], ot[:]
                )

    return nc


def _get_program():
    global _PROGRAM
    if _PROGRAM is None:
        _PROGRAM = _build_program()
    return _PROGRAM


def kernel(x, Wq, bq, Wk, bk, Wv, bv, E):
    global LAST_RESULTS
    x = np.ascontiguousarray(np.asarray(x, dtype=np.float32))
    Wq = np.asarray(Wq, dtype=np.float32)
    bq = np.asarray(bq, dtype=np.float32)
    Wk = np.asarray(Wk, dtype=np.float32)
    bk = np.asarray(bk, dtype=np.float32)
    Wv = np.asarray(Wv, dtype=np.float32)
    bv = np.asarray(bv, dtype=np.float32)
    E = np.asarray(E, dtype=np.float32)

    bf16 = mybir.dt.np(BF16)
    scale = 1.0 / math.sqrt(DH)
    xTs = [np.ascontiguousarray(x[b].T.astype(bf16)) for b in range(B)]
    in_maps = []
    for core in range(NCORES):
        b = core % B
        hg = core // B
        js = slice(hg * J, (hg + 1) * J)
        hs = slice(hg * H, (hg + 1) * H)
        wqTs = np.ascontiguousarray((Wq[js, :] * scale).T.astype(bf16))
        wkTs = np.ascontiguousarray(Wk[js, :].T.astype(bf16))
        wvTs = np.ascontiguousarray(Wv[js, :].T.astype(bf16))
        bqTs = np.ascontiguousarray((bq[js] * scale).reshape(JT, P).T)
        bkBs = np.ascontiguousarray(np.broadcast_to(bk[js], (P, J)))
        bvBs = np.ascontiguousarray(np.broadcast_to(bv[js], (P, J)))
        E_s = E[hs]  # [H, KK, L]
        eTs = np.ascontiguousarray(
            E_s.reshape(H, KK, NLC, LT4, P).transpose(2, 4, 0, 3, 1).astype(bf16)
        )  # [NLC, P, H, LT4, KK] bf16
        in_maps.append(
            {
                "xT": xTs[b],
                "wqT": wqTs,
                "wkT": wkTs,
                "wvT": wvTs,
                "bqT": bqTs,
                "bkB": bkBs,
                "bvB": bvBs,
                "eT": eTs,
            }
        )

    nc = _get_program()
    res = run_bass_kernel_spmd(nc, in_maps, list(range(NCORES)), trace=TRACE)
    LAST_RESULTS = res

    outp = np.empty((B, L, D), dtype=np.float32)
    for core in range(NCORES):
        b = core % B
        hg = core // B
        outp[b, :, hg * J : (hg + 1) * J] = res.results[core]["out"]
    return outp
